# revision 43
# baseline (speedup 1.0000x reference)
"""Multi-head attention on 8 TRN2 NeuronCores.

Sharding: core c handles batch b = c // 4 and heads [4g, 4g+4) with g = c % 4.
Each core computes its 4 heads' contribution to out[b] = concat(heads) @ W_o;
the host sums the 4 per-batch partials and adds b_o.

v8 dataflow (per core), f16 value path:
  - attn@V operand swap: expr (exp'd scores, [k, q] layout) is the STATIONARY
    matmul operand per (q-tile, k-tile); V (+ ones column for the softmax
    denominators) is the 65-wide MOVING operand.  PE time for attn@V drops
    ~8x vs streaming expr (the PE charges by moving free size only).
    u psum is [q=128, 8 q-tiles, 65] per head; column 64 = denominators.
  - normalize on the Pool engine (tensor_scalar_mul by the DVE-computed f16
    reciprocal of psum column 64), writing [q, e] f16 pair-staging tiles;
    a DMA xbar transpose ([128,128] per q-tile, 2 heads packed) produces the
    [e, q] layout the output projection needs.  No partition_broadcast.
  - one-g software pipelining: attn@V for group g is emitted after scores of
    group g+1 so the PE never waits on the ACT exp.
  - mask-mul split DVE(3)/Pool(1) per group; V psum->SBUF staging on Pool.
  - xv loaded in four 512-column slices so V projection chunks can start
    before the full xv transfer lands; explicit SP-queue DMA priority order:
    wq, xq, wk, xk, wv, mask(g0), xv(c0), mask(g1), xv(c1..c3), mask(g2,g3),
    wo, mask half 1.
  - projections f16, biases folded as in v7; warm-up matmuls ramp the PE
    p-state before the projections.
"""

import os
import numpy as np

B = 2
S = 2048
D = 1024
H = 16
DH = 64
NCORES = 8
HPC = 4  # heads per core
SH = S // 2  # s-half processed per attention sweep
TT = S // 128  # 16 t-tiles
KT = 8  # k-tiles in the contraction (1024 = 8 * 128)
G = 4  # t-tiles per softmax group (exp FD = G*1024)

_cache = {}


def _build_program():
    import concourse.mybir as mybir
    import concourse.tile as tile
    from concourse import bacc

    f32 = mybir.dt.float32
    f16 = mybir.dt.float16
    Exp = mybir.ActivationFunctionType.Exp
    Ident = mybir.ActivationFunctionType.Identity
    Alu = mybir.AluOpType

    nc = bacc.Bacc(None, target_bir_lowering=False, debug=False)
    xq = nc.declare_dram_parameter("xq", [D, S], f16, isOutput=False)
    xk = nc.declare_dram_parameter("xk", [D, S], f16, isOutput=False)
    xv = nc.declare_dram_parameter("xv", [D, S], f16, isOutput=False)
    maskT = nc.declare_dram_parameter("maskT", [S, S], f16, isOutput=False)
    wq = nc.declare_dram_parameter("wq", [2, KT, 128, 128], f16, isOutput=False)
    wk = nc.declare_dram_parameter("wk", [2, KT, 128, 128], f16, isOutput=False)
    wv = nc.declare_dram_parameter("wv", [KT, 128, 256], f16, isOutput=False)
    bqkv = nc.declare_dram_parameter("bqkv", [128, 6], f32, isOutput=False)
    bvrow = nc.declare_dram_parameter("bvrow", [1, 256], f16, isOutput=False)
    wo = nc.declare_dram_parameter("wo", [2, 128, D], f16, isOutput=False)
    out = nc.declare_dram_parameter("out", [S, D], f16, isOutput=True)

    with tile.TileContext(nc) as tc:
        with (
            tc.tile_pool(name="persist", bufs=1) as pw,
            tc.tile_pool(name="stage", bufs=1) as st,
            tc.tile_pool(name="attn", bufs=2) as at,
        ):
            wq_sb = pw.tile([128, 2, KT, 128], f16, tag="wq_sb")
            wk_sb = pw.tile([128, 2, KT, 128], f16, tag="wk_sb")
            wv_sb = pw.tile([128, KT, 256], f16, tag="wv_sb")
            bq_sb = pw.tile([128, 6], f32, tag="bq_sb")
            wo_sb = pw.tile([128, 2, D], f16, tag="wo_sb")
            ones_r = pw.tile([1, 128], f16, tag="ones_r")
            bv_sb = pw.tile([1, 256], f16, tag="bv_sb")
            bias_m2 = pw.tile([128, 1], f32, tag="bias_m2")
            warm_rhs = pw.tile([1, 512], f16, tag="warm_rhs")
            ident128 = pw.tile([128, 128], f16, tag="ident128")
            nc.gpsimd.memset(bias_m2[:], -2.0)
            nc.gpsimd.memset(ones_r[:], 1.0)
            nc.gpsimd.memset(warm_rhs[:], 1.0)
            from concourse.masks import make_identity
            make_identity(nc, ident128[:])
            QT = pw.tile([128, 2, S], f16, tag="QT", name="QT")
            KTs = pw.tile([128, 2, S], f16, tag="KTs", name="KTs")
            V8 = pw.tile([128, TT, HPC, 65], f16, tag="V8")
            for h_ in range(HPC):
                nc.gpsimd.memset(V8[:, :, h_, 64:65], 1.0)
            mask_tiles = {}

            # ---- weight/bias loads; wq first so Q projection starts ASAP
            for p in range(2):
                nc.sync.dma_start(
                    wq_sb[:, p, :, :], wq[p].rearrange("kt p m -> p kt m")
                )
            nc.sync.dma_start(bq_sb[:, :], bqkv[:, :])
            nc.sync.dma_start(bv_sb[:, :], bvrow[:, :])

            psp_ctx = tc.tile_pool(name="ps_proj", bufs=2, space="PSUM")
            psp = psp_ctx.__enter__()
            # dependency-free warm-up matmuls: ramp the PE p-state during the
            # initial DMA wait so projections run at full clock
            warm_ps = psp.tile([128, S], f32, tag="proj", name="warm_ps")
            for wi in range(14):
                nc.tensor.matmul(
                    warm_ps[:, 0:512], ones_r[:, :], warm_rhs[:, :],
                    start=True, stop=True,
                )
            stq_ctx = tc.tile_pool(name="qkstage", bufs=1)
            stq = stq_ctx.__enter__()

            def project(x_dram, w_sb, tag, accs, pool):
                xsts = [
                    pool.tile([128, 2, S], f16, tag="xst", bufs=4, name=f"{tag}{kp}")
                    for kp in range(KT // 2)
                ]
                for kp in range(KT // 2):
                    nc.sync.dma_start(
                        xsts[kp][:],
                        x_dram[kp * 256 : (kp + 1) * 256, :].rearrange(
                            "(i p) s -> p i s", p=128
                        ),
                    )
                for p in range(2):
                    for kp in range(KT // 2):
                        for i in range(2):
                            kt = 2 * kp + i
                            for ch in range(4):
                                cs = slice(ch * 512, (ch + 1) * 512)
                                nc.tensor.matmul(
                                    accs[p][:, cs],
                                    w_sb[:, p, kt, :],
                                    xsts[kp][:, i, cs],
                                    start=(kt == 0),
                                    stop=(kt == KT - 1),
                                )
                return xsts

            # ---- Phase B: project Q then K; stage with bias fold
            for x_dram, w_sb, wbi, kind in ((xq, wq_sb, 0, "q"), (xk, wk_sb, 1, "k")):
                accs = [
                    psp.tile([128, S], f32, tag="proj", name=f"acc_{kind}{pp}")
                    for pp in range(2)
                ]
                project(x_dram, w_sb, "xst" + kind, accs, stq)
                if kind == "q":  # prefetch K weights behind the xq stages
                    for p in range(2):
                        nc.sync.dma_start(
                            wk_sb[:, p, :, :],
                            wk[p].rearrange("kt p m -> p kt m"),
                        )
                dst = QT if kind == "q" else KTs
                for pp in range(2):
                    for half in range(2):
                        hs = slice(half * SH, (half + 1) * SH)
                        if pp == 0:
                            nc.scalar.activation(
                                dst[:, pp, hs], accs[pp][:, hs], Ident,
                                bias=bq_sb[:, 2 * wbi : 2 * wbi + 1],
                                scale=1.0,
                            )
                        else:
                            nc.vector.tensor_scalar(
                                dst[:, pp, hs], accs[pp][:, hs], 1.0,
                                bq_sb[:, 2 * wbi + 1 : 2 * wbi + 2],
                                Alu.mult, Alu.add,
                            )
            psp_ctx.__exit__(None, None, None)
            stq_ctx.__exit__(None, None, None)

            # V weights, then the interleaved mask(sh0)/xv column-slice loads.
            # xv slice c feeds V-projection chunk c inside head 0; mask g0/g1
            # go ahead of the later xv slices so the DVE never starves.
            nc.sync.dma_start(wv_sb[:, :, :], wv[:].rearrange("kt k e -> k kt e"))

            def load_mask(sh, g_):
                mt = at.tile(
                    [128, G, SH], f16, tag="mask", bufs=4, name=f"mask{sh}_{g_}"
                )
                mask_tiles[(sh, g_)] = mt
                nc.sync.dma_start(
                    mt[:],
                    maskT[
                        g_ * G * 128 : (g_ + 1) * G * 128, sh * SH : (sh + 1) * SH
                    ].rearrange("(tt p) s -> p tt s", p=128),
                )

            xstv = [
                [
                    st.tile([128, 2, 512], f16, tag="xstv", bufs=16,
                            name=f"xstv{c}_{kp}")
                    for kp in range(KT // 2)
                ]
                for c in range(4)
            ]

            def load_xv_slice(c):
                for kp in range(KT // 2):
                    nc.sync.dma_start(
                        xstv[c][kp][:],
                        xv[kp * 256 : (kp + 1) * 256, c * 512 : (c + 1) * 512]
                        .rearrange("(i p) s -> p i s", p=128),
                    )

            load_mask(0, 0)
            load_xv_slice(0)
            load_mask(0, 1)
            load_xv_slice(1)
            load_xv_slice(2)
            load_xv_slice(3)
            load_mask(0, 2)
            load_mask(0, 3)
            for p in range(2):
                nc.sync.dma_start(wo_sb[:, p, :], wo[p])
            for g_ in range(TT // G):
                load_mask(1, g_)

            # ---- Phase C/D: attention + output projection per s-half ----
            with (
                tc.tile_pool(name="ps_sc", bufs=2, space="PSUM") as pssc,
                tc.tile_pool(name="ps_u", bufs=2, space="PSUM") as psu,
            ):

                def emit_v_tt(tt):
                    # V in [t, e] orientation; psum borrowed from the sc pool
                    c, tl = divmod(tt, 4)
                    vps = pssc.tile([128, 256], f32, tag="sc", bufs=3)
                    for kp in range(KT // 2):
                        for i in range(2):
                            kt = 2 * kp + i
                            nc.tensor.matmul(
                                vps[:],
                                xstv[c][kp][:, i, tl * 128 : (tl + 1) * 128],
                                wv_sb[:, kt, :],
                                start=(kt == 0),
                                stop=False,
                            )
                    nc.tensor.matmul(
                        vps[:], ones_r[:, :], bv_sb[:, :], start=False, stop=True
                    )
                    nc.vector.tensor_copy(
                        V8[:, tt, :, 0:64],
                        vps[:, :].rearrange("p (h e) -> p h e", h=HPC),
                    )

                def attnv(u_ps, h, g, expr):
                    # expr stationary per (q-tile, t-tile); [V | ones] moving.
                    # A matmul with start=True zeroes its whole 2KB psum bank,
                    # so each of the two 4-q-tile banks gets exactly one
                    # start (first MM) and one stop (last MM).
                    ua, ub = u_ps
                    for qt in range(SH // 128):
                        u = ua if qt < 4 else ub
                        ql = qt % 4
                        for i in range(G):
                            tt = g * G + i
                            nc.tensor.matmul(
                                u[:, ql, 0:65],
                                expr[:, i, qt * 128 : (qt + 1) * 128],
                                V8[:, tt, h, 0:65],
                                start=(tt == 0 and ql == 0),
                                stop=(tt == TT - 1 and ql == 3),
                            )

                ob_tiles = {}

                def phase_d_unit(sh, headsT, st_i, ch, tail=False):
                    # half-unit: 512 of the 1024 output columns of one s-tile;
                    # copies collect into a 2-s-tile group buffer so the out
                    # DMAs (625ns HWDGE enqueue each) are 4x fewer
                    s0 = sh * SH
                    cs = slice(ch * 512, (ch + 1) * 512)
                    o_ps = pssc.tile([128, 512], f32, tag="sc", bufs=3)
                    for p in range(2):
                        nc.tensor.matmul(
                            o_ps[:],
                            headsT[p][:, st_i * 128 : (st_i + 1) * 128],
                            wo_sb[:, p, cs],
                            start=(p == 0),
                            stop=(p == 1),
                        )
                    key = (sh, st_i // 2)
                    if key not in ob_tiles:
                        ob_tiles[key] = at.tile(
                            [128, 2, D], f16, tag="ob", bufs=2,
                            name=f"ob_{sh}_{st_i // 2}",
                        )
                    ob = ob_tiles[key]
                    if (2 * st_i + ch) % 2 == 0:
                        nc.scalar.copy(ob[:, st_i % 2, cs], o_ps[:])
                    else:
                        nc.vector.tensor_copy(ob[:, st_i % 2, cs], o_ps[:])
                    if st_i % 2 == 1 and ch == 1:
                        r0 = s0 + (st_i // 2) * 256
                        nc.sync.dma_start(
                            out[r0 : r0 + 256, :].rearrange(
                                "(q p) d -> p q d", p=128
                            ),
                            ob[:],
                        )

                def epilogue(sh, h, u_ps, u2, headsT, last=False):
                    # denominators -> one broadcast normalize -> transpose
                    p, hh = divmod(h, 2)
                    ua, ub = u_ps
                    nrec = at.tile([128, 8], f32, tag="nrec", bufs=2)
                    with nc.allow_low_precision(
                        "softmax denominators are O(100); f16-grade reciprocal "
                        "keeps 0.05% relative error"
                    ):
                        nc.vector.reciprocal(nrec[:, 0:4], ua[:, :, 64])
                        nc.vector.reciprocal(nrec[:, 4:8], ub[:, :, 64])
                    if hh == 0:
                        u2[p] = at.tile(
                            [128, 8, 128], f16, tag="u2", bufs=3,
                            name=f"u2_{sh}_{p}",
                        )
                    if last:
                        # per-q-tile normalize, PE transpose (the PE is idle
                        # here and DMA/HWDGE are the tail bottleneck), then
                        # that s-tile's full output projection immediately
                        for qt in range(SH // 128):
                            uq = (ua if qt < 4 else ub)[:, qt % 4, 0:64]
                            nc.vector.tensor_mul(
                                u2[p][:, qt, hh * 64 : hh * 64 + 64],
                                uq,
                                nrec[:, qt : qt + 1].broadcast_to([128, 64]),
                            )
                            tp = pssc.tile([128, 128], f16, tag="sc", bufs=3,
                                           name=f"tp_{qt}")
                            nc.tensor.transpose(
                                tp[:], u2[p][:, qt, :], ident128[:]
                            )
                            nc.vector.tensor_copy(
                                headsT[p][:, qt * 128 : (qt + 1) * 128], tp[:]
                            )
                            phase_d_unit(sh, headsT, qt, 0, tail=True)
                            phase_d_unit(sh, headsT, qt, 1, tail=True)
                        return
                    nc.vector.tensor_mul(
                        u2[p][:, 0:4, hh * 64 : hh * 64 + 64],
                        ua[:, :, 0:64],
                        nrec[:, 0:4, None].broadcast_to([128, 4, 64]),
                    )
                    nc.vector.tensor_mul(
                        u2[p][:, 4:8, hh * 64 : hh * 64 + 64],
                        ub[:, :, 0:64],
                        nrec[:, 4:8, None].broadcast_to([128, 4, 64]),
                    )
                    if hh == 1:
                        for qt in range(SH // 128):
                            nc.sync.dma_start_transpose(
                                headsT[p][:, qt * 128 : (qt + 1) * 128],
                                u2[p][:, qt, :],
                            )

                pending = []
                for sh in range(2):
                    headsT = [
                        at.tile(
                            [128, SH], f16, tag="headsT", bufs=4, name=f"hT{sh}{pp}"
                        )
                        for pp in range(2)
                    ]
                    u2 = {}
                    u_tiles = {}
                    av_q = []  # (slot, lag, h, g, expr)
                    # V projection t-tiles spread over sh0's early slots;
                    # h0's attn@V runs at lag 3 so tile tt is staged in time
                    vtt = {s: [2 * s, 2 * s + 1] for s in range(8)}
                    for slot in range(HPC * (TT // G)):
                        h, g = divmod(slot, TT // G)
                        p, hh = divmod(h, 2)
                        er = slice(hh * 64, hh * 64 + 64)
                        offl = slot % 2 == 1 or slot % 8 == 4
                        if g == 0:
                            u_tiles[h] = (
                                psu.tile([128, 4, 65], f32, tag="ua", bufs=1,
                                         name=f"ua_{sh}_{h}"),
                                psu.tile([128, 4, 65], f32, tag="ub", bufs=1,
                                         name=f"ub_{sh}_{h}"),
                            )
                        msc = at.tile([128, G, SH], f16, tag="msc", bufs=2)
                        expr = at.tile([128, G, SH], f16, tag="expr", bufs=3)
                        for i in range(G):
                            tt = g * G + i
                            sc = pssc.tile([128, SH], f32, tag="sc", bufs=3)
                            for ch in range(2):
                                cs = slice(ch * 512, (ch + 1) * 512)
                                nc.tensor.matmul(
                                    sc[:, cs],
                                    KTs[er, p, tt * 128 : (tt + 1) * 128],
                                    QT[
                                        er, p,
                                        sh * SH + ch * 512 : sh * SH + (ch + 1) * 512,
                                    ],
                                    start=True,
                                    stop=True,
                                )
                            mk = mask_tiles[(sh, g)]
                            if i == 0 and offl:
                                # DVE relief: ACT drains the psum to f16,
                                # Pool (SBUF-only on HW) applies the mask
                                sc_sb = at.tile([128, SH], f16, tag="scsb",
                                                bufs=2)
                                nc.scalar.copy(sc_sb[:], sc[:])
                                nc.gpsimd.tensor_mul(
                                    msc[:, i, :], sc_sb[:], mk[:, i, :]
                                )
                            else:
                                nc.vector.tensor_mul(
                                    msc[:, i, :], sc[:], mk[:, i, :]
                                )
                        if sh == 0:
                            for tt_ in vtt.get(slot, []):
                                emit_v_tt(tt_)
                        if offl:
                            # split exp: the DVE-masked tiles go first; the
                            # Pool-masked tile trails (attn@V lag absorbs it)
                            nc.scalar.activation(
                                expr[:, 1:4, :], msc[:, 1:4, :], Exp,
                                bias=bias_m2[:],
                            )
                            nc.scalar.activation(
                                expr[:, 0:1, :], msc[:, 0:1, :], Exp,
                                bias=bias_m2[:],
                            )
                        else:
                            nc.scalar.activation(
                                expr[:], msc[:], Exp, bias=bias_m2[:]
                            )
                        av_q.append((slot, 4 if (sh == 0 and h == 0) else 2,
                                     h, g, expr))
                        npop = 0
                        while (av_q and slot - av_q[0][0] >= av_q[0][1]
                               and npop < 2):
                            npop += 1
                            _, _, ah, ag, aexpr = av_q.pop(0)
                            attnv(u_tiles[ah], ah, ag, aexpr)
                            if ag == TT // G - 1:
                                epilogue(sh, ah, u_tiles[ah], u2, headsT)
                        if pending:
                            pending.pop(0)()
                    for _, _, ah, ag, aexpr in av_q:
                        # drain leftover units into the exp-wait gaps
                        while pending:
                            pending.pop(0)()
                        attnv(u_tiles[ah], ah, ag, aexpr)
                        if ag == TT // G - 1:
                            epilogue(sh, ah, u_tiles[ah], u2, headsT,
                                     last=(sh == 1))
                    pending = [
                        (lambda sh=sh, headsT=headsT, st_i=st_i, ch=ch:
                         phase_d_unit(sh, headsT, st_i, ch))
                        for st_i in range(SH // 128)
                        for ch in range(2)
                    ] if sh == 0 else []

    nc.finalize()
    return nc


def kernel(q, k, v, mask, W_q, b_q, W_k, b_k, W_v, b_v, W_o, b_o):
    from concourse.bass_utils import run_bass_kernel_spmd

    q = np.asarray(q, dtype=np.float32)
    k = np.asarray(k, dtype=np.float32)
    v = np.asarray(v, dtype=np.float32)
    mask = np.asarray(mask, dtype=np.float32)
    W_q = np.asarray(W_q, dtype=np.float32)
    b_q = np.asarray(b_q, dtype=np.float32)
    W_k = np.asarray(W_k, dtype=np.float32)
    b_k = np.asarray(b_k, dtype=np.float32)
    W_v = np.asarray(W_v, dtype=np.float32)
    b_v = np.asarray(b_v, dtype=np.float32)
    W_o = np.asarray(W_o, dtype=np.float32)
    b_o = np.asarray(b_o, dtype=np.float32)

    if "nc" not in _cache:
        _cache["nc"] = _build_program()
    nc = _cache["nc"]

    scale = 1.0 / np.sqrt(np.float32(DH))
    maskTh = np.ascontiguousarray((mask.T * scale).astype(np.float16))

    def xT16(x_b):  # [S, D] -> [D, S] f16
        return np.ascontiguousarray(x_b.T).astype(np.float16)

    def w16(W, heads):  # [H, D, DH] -> [2, KT, 128, 128] f16
        cols = []
        for pp in range(2):
            h0, h1 = heads[2 * pp], heads[2 * pp + 1]
            wpair = np.concatenate([W[h0], W[h1]], axis=1)  # [D, 128]
            cols.append(wpair.reshape(KT, 128, 128))
        return np.ascontiguousarray(np.stack(cols, axis=0)).astype(np.float16)

    def wv16(W, heads):  # [H, D, DH] -> [KT, 128, 256] f16
        wcat = np.concatenate([W[h] for h in heads], axis=1)  # [D, 256]
        return np.ascontiguousarray(wcat.reshape(KT, 128, 256)).astype(np.float16)

    def bcat(bvec, heads):  # [H, DH] -> [128, 2] f32 (pair-concat per column)
        return np.stack(
            [
                np.concatenate([bvec[heads[2 * pp]], bvec[heads[2 * pp + 1]]])
                for pp in range(2)
            ],
            axis=1,
        ).astype(np.float32)

    in_maps = []
    for c in range(NCORES):
        b, g = divmod(c, HPC)
        heads = list(range(HPC * g, HPC * g + HPC))
        in_maps.append(
            {
                "xq": xT16(q[b]),
                "xk": xT16(k[b]),
                "xv": xT16(v[b]),
                "maskT": maskTh,
                "wq": w16(W_q, heads),
                "wk": w16(W_k, heads),
                "wv": wv16(W_v, heads),
                "bvrow": np.ascontiguousarray(
                    np.concatenate([b_v[h] for h in heads])[None, :]
                ).astype(np.float16),
                "bqkv": np.ascontiguousarray(
                    np.concatenate(
                        [bcat(b_q, heads), bcat(b_k, heads), bcat(b_v, heads)],
                        axis=1,
                    )
                ),
                "wo": np.ascontiguousarray(
                    W_o[heads[0] * DH : (heads[-1] + 1) * DH].reshape(2, 128, D)
                ).astype(np.float16),
            }
        )

    trace = bool(int(os.environ.get("KERNEL_TRACE", "0")))
    res = run_bass_kernel_spmd(nc, in_maps, list(range(NCORES)), trace=trace)
    _cache["last_results"] = res

    full = np.zeros((B, S, D), np.float32)
    for c in range(NCORES):
        full[c // HPC] += np.asarray(res.results[c]["out"], dtype=np.float32)
    full += b_o[None, None, :]
    return full


# revision 61
# speedup vs baseline: 1.0160x; 1.0160x over previous
"""Multi-head attention on 8 TRN2 NeuronCores.

Sharding: core c handles batch b = c // 4 and heads [4g, 4g+4) with g = c % 4.
Each core computes its 4 heads' contribution to out[b] = concat(heads) @ W_o;
the host sums the 4 per-batch partials and adds b_o.

v9 dataflow (per core), f16 value path:
  - attn@V operand swap: expr (exp'd scores, [k, q] layout) is the STATIONARY
    matmul operand per (q-tile, k-tile); V (+ a ones column for the softmax
    denominators) is the 65-wide MOVING operand, cutting attn@V PE time ~8x
    (the PE charges by moving free size only).  u psum is two bank-aligned
    [q=128, 4 q-tiles, 65] tiles per head (a start=True matmul zeroes its
    whole 2KB psum bank, so each bank gets exactly one start/stop pair).
  - flat 16-slot pipeline per s-half (slot = one 4-t-tile softmax group);
    attn@V trails its group's exp by 2-3 slots so the in-order PE never
    stalls on the ACT; o-projection half-units and the V projection
    (spread 2-3 t-tiles per early slot, xv loaded in 512-column slices)
    fill the PE's leftover capacity.
  - GPSIMD cannot touch PSUM on real HW, so psum drains are DVE/ACT only:
    mask-muls on the DVE, except one tile on alternating slots routed
    ACT-copy -> Pool-multiply (with that slot's exp split 3+1 so the slower
    Pool path trails off the critical chain).  Normalize = DVE reciprocal of
    u column 64 + broadcast-AP multiply; DMA xbar transposes ([128,128] per
    q-tile, 2 heads packed) produce the [e, q] layout for the o-projection.
  - last head fuses per-q-tile: normalize -> PE transpose (identity matmul)
    -> o-projection immediately, so the tail pipelines instead of
    serializing; o-projection copies collect into 2-s-tile group buffers
    (one out-DMA per 256 rows, 4x fewer 625ns HWDGE enqueues).
  - explicit SP-queue DMA priority: wq, xq, wk, xk, wv, mask g0/g1, xv c0,
    mask g2, xv c1, mask g3, xv c2/c3, wo, mask half 1; warm-up matmuls
    ramp the PE p-state through the initial DMA wait.
"""

import os
import numpy as np

B = 2
S = 2048
D = 1024
H = 16
DH = 64
NCORES = 8
HPC = 4  # heads per core
SH = S // 2  # s-half processed per attention sweep
TT = S // 128  # 16 t-tiles
KT = 8  # k-tiles in the contraction (1024 = 8 * 128)
G = 4  # t-tiles per softmax group (exp FD = G*1024)

_cache = {}


def _build_program():
    import concourse.mybir as mybir
    import concourse.tile as tile
    from concourse import bacc

    f32 = mybir.dt.float32
    f16 = mybir.dt.float16
    Exp = mybir.ActivationFunctionType.Exp
    Ident = mybir.ActivationFunctionType.Identity
    Alu = mybir.AluOpType

    nc = bacc.Bacc(None, target_bir_lowering=False, debug=False)
    xq = nc.declare_dram_parameter("xq", [D, S], f16, isOutput=False)
    xk = nc.declare_dram_parameter("xk", [D, S], f16, isOutput=False)
    xv = nc.declare_dram_parameter("xv", [D, S], f16, isOutput=False)
    maskT = nc.declare_dram_parameter("maskT", [S, S], f16, isOutput=False)
    wq = nc.declare_dram_parameter("wq", [2, KT, 128, 128], f16, isOutput=False)
    wk = nc.declare_dram_parameter("wk", [2, KT, 128, 128], f16, isOutput=False)
    wv = nc.declare_dram_parameter("wv", [KT, 128, 256], f16, isOutput=False)
    bqkv = nc.declare_dram_parameter("bqkv", [128, 6], f32, isOutput=False)
    bvrow = nc.declare_dram_parameter("bvrow", [1, 256], f16, isOutput=False)
    wo = nc.declare_dram_parameter("wo", [2, 128, D], f16, isOutput=False)
    out = nc.declare_dram_parameter("out", [S, D], f16, isOutput=True)

    with tile.TileContext(nc) as tc:
        with (
            tc.tile_pool(name="persist", bufs=1) as pw,
            tc.tile_pool(name="stage", bufs=1) as st,
            tc.tile_pool(name="attn", bufs=2) as at,
        ):
            wq_sb = pw.tile([128, 2, KT, 128], f16, tag="wq_sb")
            wk_sb = pw.tile([128, 2, KT, 128], f16, tag="wk_sb")
            wv_sb = pw.tile([128, KT, 256], f16, tag="wv_sb")
            bq_sb = pw.tile([128, 6], f32, tag="bq_sb")
            wo_sb = pw.tile([128, 2, D], f16, tag="wo_sb")
            ones_r = pw.tile([1, 128], f16, tag="ones_r")
            bv_sb = pw.tile([1, 256], f16, tag="bv_sb")
            bias_m2 = pw.tile([128, 1], f32, tag="bias_m2")
            warm_rhs = pw.tile([1, 512], f16, tag="warm_rhs")
            ident128 = pw.tile([128, 128], f16, tag="ident128")
            nc.gpsimd.memset(bias_m2[:], -2.0)
            nc.gpsimd.memset(ones_r[:], 1.0)
            nc.gpsimd.memset(warm_rhs[:], 1.0)
            from concourse.masks import make_identity
            make_identity(nc, ident128[:])
            QT = pw.tile([128, 2, S], f16, tag="QT", name="QT")
            KTs = pw.tile([128, 2, S], f16, tag="KTs", name="KTs")
            V8 = pw.tile([128, TT, HPC, 65], f16, tag="V8")
            for h_ in range(HPC):
                nc.gpsimd.memset(V8[:, :, h_, 64:65], 1.0)
            mask_tiles = {}

            # ---- weight/bias loads; wq first so Q projection starts ASAP
            for p in range(2):
                nc.sync.dma_start(
                    wq_sb[:, p, :, :], wq[p].rearrange("kt p m -> p kt m")
                )
            nc.sync.dma_start(bq_sb[:, :], bqkv[:, :])
            nc.sync.dma_start(bv_sb[:, :], bvrow[:, :])

            psp_ctx = tc.tile_pool(name="ps_proj", bufs=2, space="PSUM")
            psp = psp_ctx.__enter__()
            # dependency-free warm-up matmuls: ramp the PE p-state during the
            # initial DMA wait so projections run at full clock
            warm_ps = psp.tile([128, S], f32, tag="proj", name="warm_ps")
            for wi in range(14):
                nc.tensor.matmul(
                    warm_ps[:, 0:512], ones_r[:, :], warm_rhs[:, :],
                    start=True, stop=True,
                )
            stq_ctx = tc.tile_pool(name="qkstage", bufs=1)
            stq = stq_ctx.__enter__()

            def project(x_dram, w_sb, tag, accs, pool):
                xsts = [
                    pool.tile([128, 2, S], f16, tag="xst", bufs=4, name=f"{tag}{kp}")
                    for kp in range(KT // 2)
                ]
                for kp in range(KT // 2):
                    nc.sync.dma_start(
                        xsts[kp][:],
                        x_dram[kp * 256 : (kp + 1) * 256, :].rearrange(
                            "(i p) s -> p i s", p=128
                        ),
                    )
                for p in range(2):
                    for kp in range(KT // 2):
                        for i in range(2):
                            kt = 2 * kp + i
                            for ch in range(4):
                                cs = slice(ch * 512, (ch + 1) * 512)
                                nc.tensor.matmul(
                                    accs[p][:, cs],
                                    w_sb[:, p, kt, :],
                                    xsts[kp][:, i, cs],
                                    start=(kt == 0),
                                    stop=(kt == KT - 1),
                                )
                return xsts

            # ---- Phase B: project Q then K; stage with bias fold
            for x_dram, w_sb, wbi, kind in ((xq, wq_sb, 0, "q"), (xk, wk_sb, 1, "k")):
                accs = [
                    psp.tile([128, S], f32, tag="proj", name=f"acc_{kind}{pp}")
                    for pp in range(2)
                ]
                project(x_dram, w_sb, "xst" + kind, accs, stq)
                if kind == "q":  # prefetch K weights behind the xq stages
                    for p in range(2):
                        nc.sync.dma_start(
                            wk_sb[:, p, :, :],
                            wk[p].rearrange("kt p m -> p kt m"),
                        )
                dst = QT if kind == "q" else KTs
                for pp in range(2):
                    for half in range(2):
                        hs = slice(half * SH, (half + 1) * SH)
                        nc.scalar.activation(
                            dst[:, pp, hs], accs[pp][:, hs], Ident,
                            bias=bq_sb[:, 2 * wbi + pp : 2 * wbi + pp + 1],
                            scale=1.0,
                        )
            psp_ctx.__exit__(None, None, None)
            stq_ctx.__exit__(None, None, None)

            # V weights, then the interleaved mask(sh0)/xv column-slice loads.
            # xv slice c feeds V-projection chunk c inside head 0; mask g0/g1
            # go ahead of the later xv slices so the DVE never starves.
            nc.sync.dma_start(wv_sb[:, :, :], wv[:].rearrange("kt k e -> k kt e"))

            def load_mask(sh, g_):
                mt = at.tile(
                    [128, G, SH], f16, tag="mask", bufs=4, name=f"mask{sh}_{g_}"
                )
                mask_tiles[(sh, g_)] = mt
                nc.sync.dma_start(
                    mt[:],
                    maskT[
                        g_ * G * 128 : (g_ + 1) * G * 128, sh * SH : (sh + 1) * SH
                    ].rearrange("(tt p) s -> p tt s", p=128),
                )

            xstv = [
                [
                    st.tile([128, 2, 512], f16, tag="xstv", bufs=16,
                            name=f"xstv{c}_{kp}")
                    for kp in range(KT // 2)
                ]
                for c in range(4)
            ]

            def load_xv_slice(c):
                for kp in range(KT // 2):
                    nc.sync.dma_start(
                        xstv[c][kp][:],
                        xv[kp * 256 : (kp + 1) * 256, c * 512 : (c + 1) * 512]
                        .rearrange("(i p) s -> p i s", p=128),
                    )

            load_mask(0, 0)
            load_mask(0, 1)
            load_xv_slice(0)
            load_mask(0, 2)
            load_xv_slice(1)
            load_mask(0, 3)
            load_xv_slice(2)
            load_xv_slice(3)
            for p in range(2):
                nc.sync.dma_start(wo_sb[:, p, :], wo[p])
            for g_ in range(TT // G):
                load_mask(1, g_)

            # ---- Phase C/D: attention + output projection per s-half ----
            with (
                tc.tile_pool(name="ps_sc", bufs=2, space="PSUM") as pssc,
                tc.tile_pool(name="ps_u", bufs=2, space="PSUM") as psu,
            ):

                def emit_v_tt(tt):
                    # V in [t, e] orientation; psum borrowed from the sc pool
                    c, tl = divmod(tt, 4)
                    vps = pssc.tile([128, 256], f32, tag="sc", bufs=3)
                    for kp in range(KT // 2):
                        for i in range(2):
                            kt = 2 * kp + i
                            nc.tensor.matmul(
                                vps[:],
                                xstv[c][kp][:, i, tl * 128 : (tl + 1) * 128],
                                wv_sb[:, kt, :],
                                start=(kt == 0),
                                stop=False,
                            )
                    nc.tensor.matmul(
                        vps[:], ones_r[:, :], bv_sb[:, :], start=False, stop=True
                    )
                    nc.vector.tensor_copy(
                        V8[:, tt, :, 0:64],
                        vps[:, :].rearrange("p (h e) -> p h e", h=HPC),
                    )

                def attnv(u_ps, h, g, expr):
                    # expr stationary per (q-tile, t-tile); [V | ones] moving.
                    # A matmul with start=True zeroes its whole 2KB psum bank,
                    # so each of the two 4-q-tile banks gets exactly one
                    # start (first MM) and one stop (last MM).
                    ua, ub = u_ps
                    for qt in range(SH // 128):
                        u = ua if qt < 4 else ub
                        ql = qt % 4
                        for i in range(G):
                            tt = g * G + i
                            nc.tensor.matmul(
                                u[:, ql, 0:65],
                                expr[:, i, qt * 128 : (qt + 1) * 128],
                                V8[:, tt, h, 0:65],
                                start=(tt == 0 and ql == 0),
                                stop=(tt == TT - 1 and ql == 3),
                            )

                ob_tiles = {}

                def phase_d_unit(sh, headsT, st_i, ch, tail=False):
                    # half-unit: 512 of the 1024 output columns of one s-tile;
                    # copies collect into a 2-s-tile group buffer so the out
                    # DMAs (625ns HWDGE enqueue each) are 4x fewer
                    s0 = sh * SH
                    cs = slice(ch * 512, (ch + 1) * 512)
                    o_ps = pssc.tile([128, 512], f32, tag="sc", bufs=3)
                    for p in range(2):
                        nc.tensor.matmul(
                            o_ps[:],
                            headsT[p][:, st_i * 128 : (st_i + 1) * 128],
                            wo_sb[:, p, cs],
                            start=(p == 0),
                            stop=(p == 1),
                        )
                    key = (sh, st_i // 2)
                    if key not in ob_tiles:
                        ob_tiles[key] = at.tile(
                            [128, 2, D], f16, tag="ob", bufs=2,
                            name=f"ob_{sh}_{st_i // 2}",
                        )
                    ob = ob_tiles[key]
                    if (2 * st_i + ch) % 2 == 0:
                        nc.scalar.copy(ob[:, st_i % 2, cs], o_ps[:])
                    else:
                        nc.vector.tensor_copy(ob[:, st_i % 2, cs], o_ps[:])
                    if st_i % 2 == 1 and ch == 1:
                        r0 = s0 + (st_i // 2) * 256
                        nc.sync.dma_start(
                            out[r0 : r0 + 256, :].rearrange(
                                "(q p) d -> p q d", p=128
                            ),
                            ob[:],
                        )

                def epilogue(sh, h, u_ps, u2, headsT, last=False):
                    # denominators -> one broadcast normalize -> transpose
                    p, hh = divmod(h, 2)
                    ua, ub = u_ps
                    nrec = at.tile([128, 8], f32, tag="nrec", bufs=2)
                    with nc.allow_low_precision(
                        "softmax denominators are O(100); f16-grade reciprocal "
                        "keeps 0.05% relative error"
                    ):
                        nc.vector.reciprocal(nrec[:, 0:4], ua[:, :, 64])
                        nc.vector.reciprocal(nrec[:, 4:8], ub[:, :, 64])
                    if hh == 0:
                        u2[p] = at.tile(
                            [128, 8, 128], f16, tag="u2", bufs=3,
                            name=f"u2_{sh}_{p}",
                        )
                    if last:
                        # per-q-tile normalize, PE transpose (the PE is idle
                        # here and DMA/HWDGE are the tail bottleneck), then
                        # that s-tile's full output projection immediately
                        for qt in range(SH // 128):
                            uq = (ua if qt < 4 else ub)[:, qt % 4, 0:64]
                            nc.vector.tensor_mul(
                                u2[p][:, qt, hh * 64 : hh * 64 + 64],
                                uq,
                                nrec[:, qt : qt + 1].broadcast_to([128, 64]),
                            )
                            tp = pssc.tile([128, 128], f16, tag="sc", bufs=3,
                                           name=f"tp_{qt}")
                            nc.tensor.transpose(
                                tp[:], u2[p][:, qt, :], ident128[:]
                            )
                            nc.vector.tensor_copy(
                                headsT[p][:, qt * 128 : (qt + 1) * 128], tp[:]
                            )
                            phase_d_unit(sh, headsT, qt, 0, tail=True)
                            phase_d_unit(sh, headsT, qt, 1, tail=True)
                        return
                    nc.vector.tensor_mul(
                        u2[p][:, 0:4, hh * 64 : hh * 64 + 64],
                        ua[:, :, 0:64],
                        nrec[:, 0:4, None].broadcast_to([128, 4, 64]),
                    )
                    nc.vector.tensor_mul(
                        u2[p][:, 4:8, hh * 64 : hh * 64 + 64],
                        ub[:, :, 0:64],
                        nrec[:, 4:8, None].broadcast_to([128, 4, 64]),
                    )
                    if hh == 1:
                        for qt in range(SH // 128):
                            nc.sync.dma_start_transpose(
                                headsT[p][:, qt * 128 : (qt + 1) * 128],
                                u2[p][:, qt, :],
                            )

                pending = []
                for sh in range(2):
                    headsT = [
                        at.tile(
                            [128, SH], f16, tag="headsT", bufs=4, name=f"hT{sh}{pp}"
                        )
                        for pp in range(2)
                    ]
                    u2 = {}
                    u_tiles = {}
                    av_q = []  # (slot, lag, h, g, expr)
                    # V projection t-tiles spread over sh0's early slots;
                    # h0's attn@V runs at lag 3 so tile tt is staged in time
                    vtt = {0: [0, 1], 1: [2, 3], 2: [4, 5], 3: [6, 7],
                           4: [8, 9], 5: [10, 11, 12], 6: [13, 14, 15]}
                    for slot in range(HPC * (TT // G)):
                        h, g = divmod(slot, TT // G)
                        p, hh = divmod(h, 2)
                        er = slice(hh * 64, hh * 64 + 64)
                        offl = slot % 2 == 1
                        if g == 0:
                            u_tiles[h] = (
                                psu.tile([128, 4, 65], f32, tag="ua", bufs=1,
                                         name=f"ua_{sh}_{h}"),
                                psu.tile([128, 4, 65], f32, tag="ub", bufs=1,
                                         name=f"ub_{sh}_{h}"),
                            )
                        msc = at.tile([128, G, SH], f16, tag="msc", bufs=2)
                        expr = at.tile([128, G, SH], f16, tag="expr", bufs=3)
                        for i in range(G):
                            tt = g * G + i
                            sc = pssc.tile([128, SH], f32, tag="sc", bufs=3)
                            for ch in range(2):
                                cs = slice(ch * 512, (ch + 1) * 512)
                                nc.tensor.matmul(
                                    sc[:, cs],
                                    KTs[er, p, tt * 128 : (tt + 1) * 128],
                                    QT[
                                        er, p,
                                        sh * SH + ch * 512 : sh * SH + (ch + 1) * 512,
                                    ],
                                    start=True,
                                    stop=True,
                                )
                            mk = mask_tiles[(sh, g)]
                            if i == 0 and offl:
                                # DVE relief: ACT drains the psum to f16,
                                # Pool (SBUF-only on HW) applies the mask
                                sc_sb = at.tile([128, SH], f16, tag="scsb",
                                                bufs=2)
                                nc.scalar.copy(sc_sb[:], sc[:])
                                nc.gpsimd.tensor_mul(
                                    msc[:, i, :], sc_sb[:], mk[:, i, :]
                                )
                            else:
                                nc.vector.tensor_mul(
                                    msc[:, i, :], sc[:], mk[:, i, :]
                                )
                        if sh == 0:
                            for tt_ in vtt.get(slot, []):
                                emit_v_tt(tt_)
                        if offl:
                            # split exp: the DVE-masked tiles go first; the
                            # Pool-masked tile trails (attn@V lag absorbs it)
                            nc.scalar.activation(
                                expr[:, 1:4, :], msc[:, 1:4, :], Exp,
                                bias=bias_m2[:],
                            )
                            nc.scalar.activation(
                                expr[:, 0:1, :], msc[:, 0:1, :], Exp,
                                bias=bias_m2[:],
                            )
                        else:
                            nc.scalar.activation(
                                expr[:], msc[:], Exp, bias=bias_m2[:]
                            )
                        av_q.append((slot, 3 if (sh == 0 and h == 0) else 2,
                                     h, g, expr))
                        npop = 0
                        while (av_q and slot - av_q[0][0] >= av_q[0][1]
                               and npop < 2):
                            npop += 1
                            _, _, ah, ag, aexpr = av_q.pop(0)
                            attnv(u_tiles[ah], ah, ag, aexpr)
                            if ag == TT // G - 1:
                                epilogue(sh, ah, u_tiles[ah], u2, headsT)
                        if pending:
                            pending.pop(0)()
                    for _, _, ah, ag, aexpr in av_q:
                        # drain leftover units into the exp-wait gaps
                        while pending:
                            pending.pop(0)()
                        attnv(u_tiles[ah], ah, ag, aexpr)
                        if ag == TT // G - 1:
                            epilogue(sh, ah, u_tiles[ah], u2, headsT,
                                     last=(sh == 1))
                    pending = [
                        (lambda sh=sh, headsT=headsT, st_i=st_i, ch=ch:
                         phase_d_unit(sh, headsT, st_i, ch))
                        for st_i in range(SH // 128)
                        for ch in range(2)
                    ] if sh == 0 else []

    nc.finalize()
    return nc


def kernel(q, k, v, mask, W_q, b_q, W_k, b_k, W_v, b_v, W_o, b_o):
    from concourse.bass_utils import run_bass_kernel_spmd

    q = np.asarray(q, dtype=np.float32)
    k = np.asarray(k, dtype=np.float32)
    v = np.asarray(v, dtype=np.float32)
    mask = np.asarray(mask, dtype=np.float32)
    W_q = np.asarray(W_q, dtype=np.float32)
    b_q = np.asarray(b_q, dtype=np.float32)
    W_k = np.asarray(W_k, dtype=np.float32)
    b_k = np.asarray(b_k, dtype=np.float32)
    W_v = np.asarray(W_v, dtype=np.float32)
    b_v = np.asarray(b_v, dtype=np.float32)
    W_o = np.asarray(W_o, dtype=np.float32)
    b_o = np.asarray(b_o, dtype=np.float32)

    if "nc" not in _cache:
        _cache["nc"] = _build_program()
    nc = _cache["nc"]

    scale = 1.0 / np.sqrt(np.float32(DH))
    maskTh = np.ascontiguousarray((mask.T * scale).astype(np.float16))

    def xT16(x_b):  # [S, D] -> [D, S] f16
        return np.ascontiguousarray(x_b.T).astype(np.float16)

    def w16(W, heads):  # [H, D, DH] -> [2, KT, 128, 128] f16
        cols = []
        for pp in range(2):
            h0, h1 = heads[2 * pp], heads[2 * pp + 1]
            wpair = np.concatenate([W[h0], W[h1]], axis=1)  # [D, 128]
            cols.append(wpair.reshape(KT, 128, 128))
        return np.ascontiguousarray(np.stack(cols, axis=0)).astype(np.float16)

    def wv16(W, heads):  # [H, D, DH] -> [KT, 128, 256] f16
        wcat = np.concatenate([W[h] for h in heads], axis=1)  # [D, 256]
        return np.ascontiguousarray(wcat.reshape(KT, 128, 256)).astype(np.float16)

    def bcat(bvec, heads):  # [H, DH] -> [128, 2] f32 (pair-concat per column)
        return np.stack(
            [
                np.concatenate([bvec[heads[2 * pp]], bvec[heads[2 * pp + 1]]])
                for pp in range(2)
            ],
            axis=1,
        ).astype(np.float32)

    in_maps = []
    for c in range(NCORES):
        b, g = divmod(c, HPC)
        heads = list(range(HPC * g, HPC * g + HPC))
        in_maps.append(
            {
                "xq": xT16(q[b]),
                "xk": xT16(k[b]),
                "xv": xT16(v[b]),
                "maskT": maskTh,
                "wq": w16(W_q, heads),
                "wk": w16(W_k, heads),
                "wv": wv16(W_v, heads),
                "bvrow": np.ascontiguousarray(
                    np.concatenate([b_v[h] for h in heads])[None, :]
                ).astype(np.float16),
                "bqkv": np.ascontiguousarray(
                    np.concatenate(
                        [bcat(b_q, heads), bcat(b_k, heads), bcat(b_v, heads)],
                        axis=1,
                    )
                ),
                "wo": np.ascontiguousarray(
                    W_o[heads[0] * DH : (heads[-1] + 1) * DH].reshape(2, 128, D)
                ).astype(np.float16),
            }
        )

    trace = bool(int(os.environ.get("KERNEL_TRACE", "0")))
    res = run_bass_kernel_spmd(nc, in_maps, list(range(NCORES)), trace=trace)
    _cache["last_results"] = res

    full = np.zeros((B, S, D), np.float32)
    for c in range(NCORES):
        full[c // HPC] += np.asarray(res.results[c]["out"], dtype=np.float32)
    full += b_o[None, None, :]
    return full


# revision 71
# speedup vs baseline: 1.0196x; 1.0036x over previous
"""Multi-head attention on 8 TRN2 NeuronCores.

Sharding: core c handles batch b = c // 4 and heads [4g, 4g+4) with g = c % 4.
Each core computes its 4 heads' contribution to out[b] = concat(heads) @ W_o;
the host sums the 4 per-batch partials and adds b_o.

v9 dataflow (per core), f16 value path:
  - attn@V operand swap: expr (exp'd scores, [k, q] layout) is the STATIONARY
    matmul operand per (q-tile, k-tile); V (+ a ones column for the softmax
    denominators) is the 65-wide MOVING operand, cutting attn@V PE time ~8x
    (the PE charges by moving free size only).  u psum is two bank-aligned
    [q=128, 4 q-tiles, 65] tiles per head (a start=True matmul zeroes its
    whole 2KB psum bank, so each bank gets exactly one start/stop pair).
  - flat 16-slot pipeline per s-half (slot = one 4-t-tile softmax group);
    attn@V trails its group's exp by 2-3 slots so the in-order PE never
    stalls on the ACT; o-projection half-units and the V projection
    (spread 2-3 t-tiles per early slot, xv loaded in 512-column slices)
    fill the PE's leftover capacity.
  - GPSIMD cannot touch PSUM on real HW, so psum drains are DVE/ACT only:
    mask-muls on the DVE, except one tile on alternating slots routed
    ACT-copy -> Pool-multiply (with that slot's exp split 3+1 so the slower
    Pool path trails off the critical chain).  Normalize = DVE reciprocal of
    u column 64 + broadcast-AP multiply; DMA xbar transposes ([128,128] per
    q-tile, 2 heads packed) produce the [e, q] layout for the o-projection.
  - last head fuses per-q-tile: normalize -> PE transpose (identity matmul)
    -> o-projection immediately, so the tail pipelines instead of
    serializing; o-projection copies collect into 2-s-tile group buffers
    (one out-DMA per 256 rows, 4x fewer 625ns HWDGE enqueues).
  - explicit SP-queue DMA priority: wq, xq, wk, xk, wv, mask g0/g1, xv c0,
    mask g2, xv c1, mask g3, xv c2/c3, wo, mask half 1; warm-up matmuls
    ramp the PE p-state through the initial DMA wait.
"""

import os
import numpy as np

B = 2
S = 2048
D = 1024
H = 16
DH = 64
NCORES = 8
HPC = 4  # heads per core
SH = S // 2  # s-half processed per attention sweep
TT = S // 128  # 16 t-tiles
KT = 8  # k-tiles in the contraction (1024 = 8 * 128)
G = 4  # t-tiles per softmax group (exp FD = G*1024)

_cache = {}


def _build_program():
    import concourse.mybir as mybir
    import concourse.tile as tile
    from concourse import bacc

    f32 = mybir.dt.float32
    f16 = mybir.dt.float16
    Exp = mybir.ActivationFunctionType.Exp
    Ident = mybir.ActivationFunctionType.Identity
    Alu = mybir.AluOpType

    nc = bacc.Bacc(None, target_bir_lowering=False, debug=False)
    xq = nc.declare_dram_parameter("xq", [D, S], f16, isOutput=False)
    xk = nc.declare_dram_parameter("xk", [D, S], f16, isOutput=False)
    xv = nc.declare_dram_parameter("xv", [D, S], f16, isOutput=False)
    maskT = nc.declare_dram_parameter("maskT", [S, S], f16, isOutput=False)
    wq = nc.declare_dram_parameter("wq", [2, KT, 128, 128], f16, isOutput=False)
    wk = nc.declare_dram_parameter("wk", [2, KT, 128, 128], f16, isOutput=False)
    wv = nc.declare_dram_parameter("wv", [KT, 128, 256], f16, isOutput=False)
    bqkv = nc.declare_dram_parameter("bqkv", [128, 6], f32, isOutput=False)
    bvrow = nc.declare_dram_parameter("bvrow", [1, 256], f16, isOutput=False)
    wo = nc.declare_dram_parameter("wo", [2, 128, D], f16, isOutput=False)
    out = nc.declare_dram_parameter("out", [S, D], f16, isOutput=True)

    with tile.TileContext(nc) as tc:
        with (
            tc.tile_pool(name="persist", bufs=1) as pw,
            tc.tile_pool(name="stage", bufs=1) as st,
            tc.tile_pool(name="attn", bufs=2) as at,
        ):
            wq_sb = pw.tile([128, 2, KT, 128], f16, tag="wq_sb")
            wk_sb = pw.tile([128, 2, KT, 128], f16, tag="wk_sb")
            wv_sb = pw.tile([128, KT, 256], f16, tag="wv_sb")
            bq_sb = pw.tile([128, 6], f32, tag="bq_sb")
            wo_sb = pw.tile([128, 2, D], f16, tag="wo_sb")
            ones_r = pw.tile([1, 128], f16, tag="ones_r")
            bv_sb = pw.tile([1, 256], f16, tag="bv_sb")
            bias_m2 = pw.tile([128, 1], f32, tag="bias_m2")
            warm_rhs = pw.tile([1, 512], f16, tag="warm_rhs")
            ident128 = pw.tile([128, 128], f16, tag="ident128")
            nc.gpsimd.memset(bias_m2[:], -2.0)
            nc.gpsimd.memset(ones_r[:], 1.0)
            nc.gpsimd.memset(warm_rhs[:], 1.0)
            from concourse.masks import make_identity
            make_identity(nc, ident128[:])
            QT = pw.tile([128, 2, S], f16, tag="QT", name="QT")
            KTs = pw.tile([128, 2, S], f16, tag="KTs", name="KTs")
            V8 = pw.tile([128, TT, HPC, 65], f16, tag="V8")
            for h_ in range(HPC):
                nc.gpsimd.memset(V8[:, :, h_, 64:65], 1.0)
            mask_tiles = {}

            # ---- weight/bias loads; wq first so Q projection starts ASAP
            for p in range(2):
                nc.sync.dma_start(
                    wq_sb[:, p, :, :], wq[p].rearrange("kt p m -> p kt m")
                )
            nc.sync.dma_start(bq_sb[:, :], bqkv[:, :])
            nc.sync.dma_start(bv_sb[:, :], bvrow[:, :])

            psp_ctx = tc.tile_pool(name="ps_proj", bufs=2, space="PSUM")
            psp = psp_ctx.__enter__()
            # dependency-free warm-up matmuls: ramp the PE p-state during the
            # initial DMA wait so projections run at full clock
            warm_ps = psp.tile([128, S], f32, tag="proj", name="warm_ps")
            for wi in range(14):
                nc.tensor.matmul(
                    warm_ps[:, 0:512], ones_r[:, :], warm_rhs[:, :],
                    start=True, stop=True,
                )
            stq_ctx = tc.tile_pool(name="qkstage", bufs=1)
            stq = stq_ctx.__enter__()

            def project(x_dram, w_sb, tag, accs, pool, kt_major=False):
                xsts = [
                    pool.tile([128, 2, S], f16, tag="xst", bufs=4, name=f"{tag}{kp}")
                    for kp in range(KT // 2)
                ]
                for kp in range(KT // 2):
                    nc.sync.dma_start(
                        xsts[kp][:],
                        x_dram[kp * 256 : (kp + 1) * 256, :].rearrange(
                            "(i p) s -> p i s", p=128
                        ),
                    )
                if kt_major:
                    # both pairs complete with the last x chunk, so the
                    # first scores are not stuck behind pair 1 on the PE
                    for kp in range(KT // 2):
                        for i in range(2):
                            kt = 2 * kp + i
                            for p in range(2):
                                for ch in range(4):
                                    cs = slice(ch * 512, (ch + 1) * 512)
                                    nc.tensor.matmul(
                                        accs[p][:, cs],
                                        w_sb[:, p, kt, :],
                                        xsts[kp][:, i, cs],
                                        start=(kt == 0),
                                        stop=(kt == KT - 1),
                                    )
                    return xsts
                for p in range(2):
                    for kp in range(KT // 2):
                        for i in range(2):
                            kt = 2 * kp + i
                            for ch in range(4):
                                cs = slice(ch * 512, (ch + 1) * 512)
                                nc.tensor.matmul(
                                    accs[p][:, cs],
                                    w_sb[:, p, kt, :],
                                    xsts[kp][:, i, cs],
                                    start=(kt == 0),
                                    stop=(kt == KT - 1),
                                )
                return xsts

            # ---- Phase B: project Q then K; stage with bias fold
            for x_dram, w_sb, wbi, kind in ((xq, wq_sb, 0, "q"), (xk, wk_sb, 1, "k")):
                accs = [
                    psp.tile([128, S], f32, tag="proj", name=f"acc_{kind}{pp}")
                    for pp in range(2)
                ]
                project(x_dram, w_sb, "xst" + kind, accs, stq,
                        kt_major=False)
                if kind == "q":  # prefetch K weights behind the xq stages
                    for p in range(2):
                        nc.sync.dma_start(
                            wk_sb[:, p, :, :],
                            wk[p].rearrange("kt p m -> p kt m"),
                        )
                dst = QT if kind == "q" else KTs
                for pp in range(2):
                    for half in range(2):
                        hs = slice(half * SH, (half + 1) * SH)
                        nc.scalar.activation(
                            dst[:, pp, hs], accs[pp][:, hs], Ident,
                            bias=bq_sb[:, 2 * wbi + pp : 2 * wbi + pp + 1],
                            scale=1.0,
                        )
            psp_ctx.__exit__(None, None, None)
            stq_ctx.__exit__(None, None, None)

            # V weights, then the interleaved mask(sh0)/xv column-slice loads.
            # xv slice c feeds V-projection chunk c inside head 0; mask g0/g1
            # go ahead of the later xv slices so the DVE never starves.
            nc.sync.dma_start(wv_sb[:, :, :], wv[:].rearrange("kt k e -> k kt e"))

            def load_mask(sh, g_):
                mt = at.tile(
                    [128, G, SH], f16, tag="mask", bufs=4, name=f"mask{sh}_{g_}"
                )
                mask_tiles[(sh, g_)] = mt
                nc.sync.dma_start(
                    mt[:],
                    maskT[
                        g_ * G * 128 : (g_ + 1) * G * 128, sh * SH : (sh + 1) * SH
                    ].rearrange("(tt p) s -> p tt s", p=128),
                )

            xstv = [
                [
                    st.tile([128, 2, 512], f16, tag="xstv", bufs=16,
                            name=f"xstv{c}_{kp}")
                    for kp in range(KT // 2)
                ]
                for c in range(4)
            ]

            def load_xv_slice(c):
                for kp in range(KT // 2):
                    nc.sync.dma_start(
                        xstv[c][kp][:],
                        xv[kp * 256 : (kp + 1) * 256, c * 512 : (c + 1) * 512]
                        .rearrange("(i p) s -> p i s", p=128),
                    )

            load_mask(0, 0)
            load_mask(0, 1)
            load_xv_slice(0)
            load_mask(0, 2)
            load_xv_slice(1)
            load_mask(0, 3)
            load_xv_slice(2)
            load_xv_slice(3)
            for p in range(2):
                nc.sync.dma_start(wo_sb[:, p, :], wo[p])
            for g_ in range(TT // G):
                load_mask(1, g_)

            # ---- Phase C/D: attention + output projection per s-half ----
            with (
                tc.tile_pool(name="ps_sc", bufs=2, space="PSUM") as pssc,
                tc.tile_pool(name="ps_u", bufs=2, space="PSUM") as psu,
            ):

                def emit_v_tt(tt):
                    # V in [t, e] orientation; psum borrowed from the sc pool
                    c, tl = divmod(tt, 4)
                    vps = pssc.tile([128, 256], f32, tag="sc", bufs=3)
                    for kp in range(KT // 2):
                        for i in range(2):
                            kt = 2 * kp + i
                            nc.tensor.matmul(
                                vps[:],
                                xstv[c][kp][:, i, tl * 128 : (tl + 1) * 128],
                                wv_sb[:, kt, :],
                                start=(kt == 0),
                                stop=False,
                            )
                    nc.tensor.matmul(
                        vps[:], ones_r[:, :], bv_sb[:, :], start=False, stop=True
                    )
                    nc.vector.tensor_copy(
                        V8[:, tt, :, 0:64],
                        vps[:, :].rearrange("p (h e) -> p h e", h=HPC),
                    )

                def attnv(u_ps, h, g, expr, offl_g=False):
                    # expr stationary per (q-tile, t-tile); [V | ones] moving.
                    # A matmul with start=True zeroes its whole 2KB psum bank,
                    # so each of the two 4-q-tile banks gets exactly one
                    # start (first MM) and one stop (last MM).
                    ua, ub = u_ps
                    iorder = (1, 2, 3, 0) if offl_g else tuple(range(G))
                    for qt in range(SH // 128):
                        u = ua if qt < 4 else ub
                        ql = qt % 4
                        for i in iorder:
                            tt = g * G + i
                            nc.tensor.matmul(
                                u[:, ql, 0:65],
                                expr[:, i, qt * 128 : (qt + 1) * 128],
                                V8[:, tt, h, 0:65],
                                start=(g == 0 and i == iorder[0] and ql == 0),
                                stop=(g == TT // G - 1 and i == iorder[-1]
                                      and ql == 3),
                            )

                ob_tiles = {}

                def phase_d_unit(sh, headsT, st_i, ch, tail=False):
                    # half-unit: 512 of the 1024 output columns of one s-tile;
                    # copies collect into a 2-s-tile group buffer so the out
                    # DMAs (625ns HWDGE enqueue each) are 4x fewer
                    s0 = sh * SH
                    cs = slice(ch * 512, (ch + 1) * 512)
                    o_ps = pssc.tile([128, 512], f32, tag="sc", bufs=3)
                    for p in range(2):
                        nc.tensor.matmul(
                            o_ps[:],
                            headsT[p][:, st_i * 128 : (st_i + 1) * 128],
                            wo_sb[:, p, cs],
                            start=(p == 0),
                            stop=(p == 1),
                        )
                    key = (sh, st_i // 2)
                    if key not in ob_tiles:
                        ob_tiles[key] = at.tile(
                            [128, 2, D], f16, tag="ob", bufs=2,
                            name=f"ob_{sh}_{st_i // 2}",
                        )
                    ob = ob_tiles[key]
                    if (2 * st_i + ch) % 2 == 0:
                        nc.scalar.copy(ob[:, st_i % 2, cs], o_ps[:])
                    else:
                        nc.vector.tensor_copy(ob[:, st_i % 2, cs], o_ps[:])
                    if st_i % 2 == 1 and ch == 1:
                        r0 = s0 + (st_i // 2) * 256
                        nc.sync.dma_start(
                            out[r0 : r0 + 256, :].rearrange(
                                "(q p) d -> p q d", p=128
                            ),
                            ob[:],
                        )

                def epilogue_b(sh, h, u_ps, u2, headsT):
                    # deferred half of the epilogue: second-bank normalize
                    # (+ its transposes), pushed one slot later to spread
                    # the DVE load at head boundaries
                    p, hh = divmod(h, 2)
                    ua, ub = u_ps
                    nrec = nrec_tiles[(sh, h)]
                    nc.vector.tensor_mul(
                        u2[p][:, 4:8, hh * 64 : hh * 64 + 64],
                        ub[:, :, 0:64],
                        nrec[:, 4:8, None].broadcast_to([128, 4, 64]),
                    )
                    if hh == 1:
                        for qt in range(4, SH // 128):
                            nc.sync.dma_start_transpose(
                                headsT[p][:, qt * 128 : (qt + 1) * 128],
                                u2[p][:, qt, :],
                            )

                def epilogue(sh, h, u_ps, u2, headsT, last=False):
                    # denominators -> one broadcast normalize -> transpose
                    p, hh = divmod(h, 2)
                    ua, ub = u_ps
                    nrec = at.tile([128, 8], f32, tag="nrec", bufs=2)
                    with nc.allow_low_precision(
                        "softmax denominators are O(100); f16-grade reciprocal "
                        "keeps 0.05% relative error"
                    ):
                        nc.vector.reciprocal(nrec[:, 0:4], ua[:, :, 64])
                        nc.vector.reciprocal(nrec[:, 4:8], ub[:, :, 64])
                    if hh == 0:
                        u2[p] = at.tile(
                            [128, 8, 128], f16, tag="u2", bufs=3,
                            name=f"u2_{sh}_{p}",
                        )
                    if last:
                        # per-q-tile normalize, PE transpose (the PE is idle
                        # here and DMA/HWDGE are the tail bottleneck), then
                        # that s-tile's full output projection immediately
                        for qt in range(SH // 128):
                            uq = (ua if qt < 4 else ub)[:, qt % 4, 0:64]
                            nc.vector.tensor_mul(
                                u2[p][:, qt, hh * 64 : hh * 64 + 64],
                                uq,
                                nrec[:, qt : qt + 1].broadcast_to([128, 64]),
                            )
                            tp = pssc.tile([128, 128], f16, tag="sc", bufs=3,
                                           name=f"tp_{qt}")
                            nc.tensor.transpose(
                                tp[:], u2[p][:, qt, :], ident128[:]
                            )
                            nc.vector.tensor_copy(
                                headsT[p][:, qt * 128 : (qt + 1) * 128], tp[:]
                            )
                            phase_d_unit(sh, headsT, qt, 0, tail=True)
                            phase_d_unit(sh, headsT, qt, 1, tail=True)
                        return
                    nrec_tiles[(sh, h)] = nrec
                    nc.vector.tensor_mul(
                        u2[p][:, 0:4, hh * 64 : hh * 64 + 64],
                        ua[:, :, 0:64],
                        nrec[:, 0:4, None].broadcast_to([128, 4, 64]),
                    )
                    if hh == 1:
                        for qt in range(4):
                            nc.sync.dma_start_transpose(
                                headsT[p][:, qt * 128 : (qt + 1) * 128],
                                u2[p][:, qt, :],
                            )

                pending = []
                nrec_tiles = {}
                for sh in range(2):
                    headsT = [
                        at.tile(
                            [128, SH], f16, tag="headsT", bufs=4, name=f"hT{sh}{pp}"
                        )
                        for pp in range(2)
                    ]
                    u2 = {}
                    u_tiles = {}
                    epi_b = []
                    av_q = []  # (slot, lag, h, g, expr)
                    # V projection t-tiles spread over sh0's early slots;
                    # h0's attn@V runs at lag 3 so tile tt is staged in time
                    vtt = {0: [0, 1], 1: [2, 3], 2: [4, 5], 3: [6, 7],
                           4: [8, 9], 5: [10, 11, 12], 6: [13, 14, 15]}
                    for slot in range(HPC * (TT // G)):
                        h, g = divmod(slot, TT // G)
                        p, hh = divmod(h, 2)
                        er = slice(hh * 64, hh * 64 + 64)
                        offl = slot % 2 == 1
                        if g == 0:
                            u_tiles[h] = (
                                psu.tile([128, 4, 65], f32, tag="ua", bufs=1,
                                         name=f"ua_{sh}_{h}"),
                                psu.tile([128, 4, 65], f32, tag="ub", bufs=1,
                                         name=f"ub_{sh}_{h}"),
                            )
                        msc = at.tile([128, G, SH], f16, tag="msc", bufs=2)
                        expr = at.tile([128, G, SH], f16, tag="expr", bufs=3)
                        for i in range(G):
                            tt = g * G + i
                            sc = pssc.tile([128, SH], f32, tag="sc", bufs=3)
                            for ch in range(2):
                                cs = slice(ch * 512, (ch + 1) * 512)
                                nc.tensor.matmul(
                                    sc[:, cs],
                                    KTs[er, p, tt * 128 : (tt + 1) * 128],
                                    QT[
                                        er, p,
                                        sh * SH + ch * 512 : sh * SH + (ch + 1) * 512,
                                    ],
                                    start=True,
                                    stop=True,
                                )
                            mk = mask_tiles[(sh, g)]
                            if i == 0 and offl:
                                # DVE relief: ACT drains the psum to f16,
                                # Pool (SBUF-only on HW) applies the mask
                                sc_sb = at.tile([128, SH], f16, tag="scsb",
                                                bufs=2)
                                nc.scalar.copy(sc_sb[:], sc[:])
                                nc.gpsimd.tensor_mul(
                                    msc[:, i, :], sc_sb[:], mk[:, i, :]
                                )
                            else:
                                nc.vector.tensor_mul(
                                    msc[:, i, :], sc[:], mk[:, i, :]
                                )
                        if sh == 0:
                            for tt_ in vtt.get(slot, []):
                                emit_v_tt(tt_)
                        if offl:
                            # split exp: the DVE-masked tiles go first; the
                            # Pool-masked tile trails (attn@V lag absorbs it)
                            nc.scalar.activation(
                                expr[:, 1:4, :], msc[:, 1:4, :], Exp,
                                bias=bias_m2[:],
                            )
                            nc.scalar.activation(
                                expr[:, 0:1, :], msc[:, 0:1, :], Exp,
                                bias=bias_m2[:],
                            )
                        else:
                            nc.scalar.activation(
                                expr[:], msc[:], Exp, bias=bias_m2[:]
                            )
                        while epi_b:
                            epi_b.pop(0)()
                        av_q.append((slot, 3 if (sh == 0 and h == 0) else 2,
                                     h, g, expr, offl))
                        npop = 0
                        while (av_q and slot - av_q[0][0] >= av_q[0][1]
                               and npop < 2):
                            npop += 1
                            _, _, ah, ag, aexpr, aoffl = av_q.pop(0)
                            attnv(u_tiles[ah], ah, ag, aexpr, aoffl)
                            if ag == TT // G - 1:
                                epilogue(sh, ah, u_tiles[ah], u2, headsT)
                                epi_b.append(
                                    lambda sh=sh, ah=ah, u=u_tiles[ah],
                                    u2=u2, hT=headsT:
                                    epilogue_b(sh, ah, u, u2, hT)
                                )
                        if pending:
                            pending.pop(0)()
                    for _, _, ah, ag, aexpr, aoffl in av_q:
                        # drain leftover units into the exp-wait gaps
                        while pending:
                            pending.pop(0)()
                        attnv(u_tiles[ah], ah, ag, aexpr, aoffl)
                        if ag == TT // G - 1:
                            epilogue(sh, ah, u_tiles[ah], u2, headsT,
                                     last=(sh == 1))
                            if sh == 0:
                                epilogue_b(sh, ah, u_tiles[ah], u2, headsT)
                    pending = [
                        (lambda sh=sh, headsT=headsT, st_i=st_i, ch=ch:
                         phase_d_unit(sh, headsT, st_i, ch))
                        for st_i in range(SH // 128)
                        for ch in range(2)
                    ] if sh == 0 else []

    nc.finalize()
    return nc


def kernel(q, k, v, mask, W_q, b_q, W_k, b_k, W_v, b_v, W_o, b_o):
    from concourse.bass_utils import run_bass_kernel_spmd

    q = np.asarray(q, dtype=np.float32)
    k = np.asarray(k, dtype=np.float32)
    v = np.asarray(v, dtype=np.float32)
    mask = np.asarray(mask, dtype=np.float32)
    W_q = np.asarray(W_q, dtype=np.float32)
    b_q = np.asarray(b_q, dtype=np.float32)
    W_k = np.asarray(W_k, dtype=np.float32)
    b_k = np.asarray(b_k, dtype=np.float32)
    W_v = np.asarray(W_v, dtype=np.float32)
    b_v = np.asarray(b_v, dtype=np.float32)
    W_o = np.asarray(W_o, dtype=np.float32)
    b_o = np.asarray(b_o, dtype=np.float32)

    if "nc" not in _cache:
        _cache["nc"] = _build_program()
    nc = _cache["nc"]

    scale = 1.0 / np.sqrt(np.float32(DH))
    maskTh = np.ascontiguousarray((mask.T * scale).astype(np.float16))

    def xT16(x_b):  # [S, D] -> [D, S] f16
        return np.ascontiguousarray(x_b.T).astype(np.float16)

    def w16(W, heads):  # [H, D, DH] -> [2, KT, 128, 128] f16
        cols = []
        for pp in range(2):
            h0, h1 = heads[2 * pp], heads[2 * pp + 1]
            wpair = np.concatenate([W[h0], W[h1]], axis=1)  # [D, 128]
            cols.append(wpair.reshape(KT, 128, 128))
        return np.ascontiguousarray(np.stack(cols, axis=0)).astype(np.float16)

    def wv16(W, heads):  # [H, D, DH] -> [KT, 128, 256] f16
        wcat = np.concatenate([W[h] for h in heads], axis=1)  # [D, 256]
        return np.ascontiguousarray(wcat.reshape(KT, 128, 256)).astype(np.float16)

    def bcat(bvec, heads):  # [H, DH] -> [128, 2] f32 (pair-concat per column)
        return np.stack(
            [
                np.concatenate([bvec[heads[2 * pp]], bvec[heads[2 * pp + 1]]])
                for pp in range(2)
            ],
            axis=1,
        ).astype(np.float32)

    in_maps = []
    for c in range(NCORES):
        b, g = divmod(c, HPC)
        heads = list(range(HPC * g, HPC * g + HPC))
        in_maps.append(
            {
                "xq": xT16(q[b]),
                "xk": xT16(k[b]),
                "xv": xT16(v[b]),
                "maskT": maskTh,
                "wq": w16(W_q, heads),
                "wk": w16(W_k, heads),
                "wv": wv16(W_v, heads),
                "bvrow": np.ascontiguousarray(
                    np.concatenate([b_v[h] for h in heads])[None, :]
                ).astype(np.float16),
                "bqkv": np.ascontiguousarray(
                    np.concatenate(
                        [bcat(b_q, heads), bcat(b_k, heads), bcat(b_v, heads)],
                        axis=1,
                    )
                ),
                "wo": np.ascontiguousarray(
                    W_o[heads[0] * DH : (heads[-1] + 1) * DH].reshape(2, 128, D)
                ).astype(np.float16),
            }
        )

    trace = bool(int(os.environ.get("KERNEL_TRACE", "0")))
    res = run_bass_kernel_spmd(nc, in_maps, list(range(NCORES)), trace=trace)
    _cache["last_results"] = res

    full = np.zeros((B, S, D), np.float32)
    for c in range(NCORES):
        full[c // HPC] += np.asarray(res.results[c]["out"], dtype=np.float32)
    full += b_o[None, None, :]
    return full


# revision 77
# speedup vs baseline: 1.0468x; 1.0266x over previous
"""Multi-head attention on 8 TRN2 NeuronCores.

Sharding: core c handles batch b = c // 4 and heads [4g, 4g+4) with g = c % 4.
Each core computes its 4 heads' contribution to out[b] = concat(heads) @ W_o;
the host sums the 4 per-batch partials and adds b_o.

v9 dataflow (per core), f16 value path:
  - attn@V operand swap: expr (exp'd scores, [k, q] layout) is the STATIONARY
    matmul operand per (q-tile, k-tile); V (+ a ones column for the softmax
    denominators) is the 65-wide MOVING operand, cutting attn@V PE time ~8x
    (the PE charges by moving free size only).  u psum is two bank-aligned
    [q=128, 4 q-tiles, 65] tiles per head (a start=True matmul zeroes its
    whole 2KB psum bank, so each bank gets exactly one start/stop pair).
  - flat 16-slot pipeline per s-half (slot = one 4-t-tile softmax group);
    attn@V trails its group's exp by 2-3 slots so the in-order PE never
    stalls on the ACT; o-projection half-units and the V projection
    (spread 2-3 t-tiles per early slot, xv loaded in 512-column slices)
    fill the PE's leftover capacity.
  - GPSIMD cannot touch PSUM on real HW, so psum drains are DVE/ACT only:
    mask-muls on the DVE, except one tile on alternating slots routed
    ACT-copy -> Pool-multiply (with that slot's exp split 3+1 so the slower
    Pool path trails off the critical chain).  Normalize = DVE reciprocal of
    u column 64 + broadcast-AP multiply; DMA xbar transposes ([128,128] per
    q-tile, 2 heads packed) produce the [e, q] layout for the o-projection.
  - last head fuses per-q-tile: normalize -> PE transpose (identity matmul)
    -> o-projection immediately, so the tail pipelines instead of
    serializing; o-projection copies collect into 2-s-tile group buffers
    (one out-DMA per 256 rows, 4x fewer 625ns HWDGE enqueues).
  - explicit SP-queue DMA priority: wq, xq, wk, xk, wv, mask g0/g1, xv c0,
    mask g2, xv c1, mask g3, xv c2/c3, wo, mask half 1; warm-up matmuls
    ramp the PE p-state through the initial DMA wait.
"""

import os
import numpy as np

B = 2
S = 2048
D = 1024
H = 16
DH = 64
NCORES = 8
HPC = 4  # heads per core
SH = S // 2  # s-half processed per attention sweep
TT = S // 128  # 16 t-tiles
KT = 8  # k-tiles in the contraction (1024 = 8 * 128)
G = 4  # t-tiles per softmax group (exp FD = G*1024)

_cache = {}


def _build_program():
    import concourse.mybir as mybir
    import concourse.tile as tile
    from concourse import bacc

    f32 = mybir.dt.float32
    f16 = mybir.dt.float16
    Exp = mybir.ActivationFunctionType.Exp
    Ident = mybir.ActivationFunctionType.Identity
    Alu = mybir.AluOpType

    nc = bacc.Bacc(None, target_bir_lowering=False, debug=False)
    xq = nc.declare_dram_parameter("xq", [D, S], f16, isOutput=False)
    xk = nc.declare_dram_parameter("xk", [D, S], f16, isOutput=False)
    xv = nc.declare_dram_parameter("xv", [D, S], f16, isOutput=False)
    maskT = nc.declare_dram_parameter("maskT", [S, S], f16, isOutput=False)
    wq = nc.declare_dram_parameter("wq", [2, KT, 128, 128], f16, isOutput=False)
    wk = nc.declare_dram_parameter("wk", [2, KT, 128, 128], f16, isOutput=False)
    wv = nc.declare_dram_parameter("wv", [KT, 128, 256], f16, isOutput=False)
    bqkv = nc.declare_dram_parameter("bqkv", [128, 6], f32, isOutput=False)
    bvrow = nc.declare_dram_parameter("bvrow", [1, 256], f16, isOutput=False)
    wo = nc.declare_dram_parameter("wo", [2, 128, D], f16, isOutput=False)
    out = nc.declare_dram_parameter("out", [S, D], f16, isOutput=True)

    with tile.TileContext(nc) as tc:
        with (
            tc.tile_pool(name="persist", bufs=1) as pw,
            tc.tile_pool(name="stage", bufs=1) as st,
            tc.tile_pool(name="attn", bufs=2) as at,
        ):
            wq_sb = pw.tile([128, 2, KT, 128], f16, tag="wq_sb")
            wk_sb = pw.tile([128, 2, KT, 128], f16, tag="wk_sb")
            wv_sb = pw.tile([128, KT, 256], f16, tag="wv_sb")
            bq_sb = pw.tile([128, 6], f32, tag="bq_sb")
            wo_sb = pw.tile([128, 2, D], f16, tag="wo_sb")
            ones_r = pw.tile([1, 128], f16, tag="ones_r")
            bv_sb = pw.tile([1, 256], f16, tag="bv_sb")
            bias_m2 = pw.tile([128, 1], f32, tag="bias_m2")
            warm_rhs = pw.tile([1, 512], f16, tag="warm_rhs")
            ident128 = pw.tile([128, 128], f16, tag="ident128")
            nc.gpsimd.memset(bias_m2[:], -2.0)
            nc.gpsimd.memset(ones_r[:], 1.0)
            nc.gpsimd.memset(warm_rhs[:], 1.0)
            from concourse.masks import make_identity
            make_identity(nc, ident128[:])
            QT = pw.tile([128, 2, S], f16, tag="QT", name="QT")
            KTs = pw.tile([128, 2, S], f16, tag="KTs", name="KTs")
            V8 = pw.tile([128, TT, HPC, 65], f16, tag="V8")
            for h_ in range(HPC):
                nc.gpsimd.memset(V8[:, :, h_, 64:65], 1.0)
            mask_tiles = {}

            # ---- weight/bias loads; wq first so Q projection starts ASAP
            for p in range(2):
                nc.sync.dma_start(
                    wq_sb[:, p, :, :], wq[p].rearrange("kt p m -> p kt m")
                )
            nc.sync.dma_start(bq_sb[:, :], bqkv[:, :])
            nc.sync.dma_start(bv_sb[:, :], bvrow[:, :])

            psp_ctx = tc.tile_pool(name="ps_proj", bufs=2, space="PSUM")
            psp = psp_ctx.__enter__()
            # dependency-free warm-up matmuls: ramp the PE p-state during the
            # initial DMA wait so projections run at full clock
            warm_ps = psp.tile([128, SH], f32, tag="proj", bufs=4,
                               name="warm_ps")
            for wi in range(14):
                nc.tensor.matmul(
                    warm_ps[:, 0:512], ones_r[:, :], warm_rhs[:, :],
                    start=True, stop=True,
                )
            stq_ctx = tc.tile_pool(name="qkstage", bufs=1)
            stq = stq_ctx.__enter__()

            def half_dmas(x_dram, wbi, half):
                xh = [
                    stq.tile([128, 2, SH], f16, tag="xst", bufs=8,
                             name=f"x{wbi}{half}_{kp}")
                    for kp in range(KT // 2)
                ]
                for kp in range(KT // 2):
                    nc.sync.dma_start(
                        xh[kp][:],
                        x_dram[
                            kp * 256 : (kp + 1) * 256,
                            half * SH : (half + 1) * SH,
                        ].rearrange("(i p) s -> p i s", p=128),
                    )
                return xh

            def half_compute(xh, w_sb, wbi, half, dst):
                accs = [
                    psp.tile([128, SH], f32, tag="proj", bufs=4,
                             name=f"acc{wbi}{half}{pp}")
                    for pp in range(2)
                ]
                for p in range(2):
                    for kp in range(KT // 2):
                        for i in range(2):
                            kt = 2 * kp + i
                            for ch in range(2):
                                cs = slice(ch * 512, (ch + 1) * 512)
                                nc.tensor.matmul(
                                    accs[p][:, cs],
                                    w_sb[:, p, kt, :],
                                    xh[kp][:, i, cs],
                                    start=(kt == 0),
                                    stop=(kt == KT - 1),
                                )
                hs = slice(half * SH, (half + 1) * SH)
                for pp in range(2):
                    nc.scalar.activation(
                        dst[:, pp, hs], accs[pp][:], Ident,
                        bias=bq_sb[:, 2 * wbi + pp : 2 * wbi + pp + 1],
                        scale=1.0,
                    )

            # ---- Phase B: Q/K in column-half units so the first scores
            # wait on ~5MB of input DMA instead of 9MB; the x stage ring
            # holds both tensors' h0 halves at once (8 x 4KB).
            def load_mask(sh, g_):
                mt = at.tile(
                    [128, G, SH], f16, tag="mask", bufs=4, name=f"mask{sh}_{g_}"
                )
                mask_tiles[(sh, g_)] = mt
                nc.sync.dma_start(
                    mt[:],
                    maskT[
                        g_ * G * 128 : (g_ + 1) * G * 128, sh * SH : (sh + 1) * SH
                    ].rearrange("(tt p) s -> p tt s", p=128),
                )

            xstv = [
                [
                    st.tile([128, 2, 512], f16, tag="xstv", bufs=16,
                            name=f"xstv{c}_{kp}")
                    for kp in range(KT // 2)
                ]
                for c in range(4)
            ]

            def load_xv_slice(c):
                for kp in range(KT // 2):
                    nc.sync.dma_start(
                        xstv[c][kp][:],
                        xv[kp * 256 : (kp + 1) * 256, c * 512 : (c + 1) * 512]
                        .rearrange("(i p) s -> p i s", p=128),
                    )

            xq_h0 = half_dmas(xq, 0, 0)
            for p in range(2):  # prefetch K weights behind the xq-h0 stages
                nc.sync.dma_start(
                    wk_sb[:, p, :, :], wk[p].rearrange("kt p m -> p kt m")
                )
            xk_h0 = half_dmas(xk, 1, 0)
            load_mask(0, 0)
            xq_h1 = half_dmas(xq, 0, 1)
            xk_h1 = half_dmas(xk, 1, 1)
            load_mask(0, 1)
            nc.sync.dma_start(wv_sb[:, :, :], wv[:].rearrange("kt k e -> k kt e"))
            load_xv_slice(0)
            load_mask(0, 2)
            load_xv_slice(1)
            load_mask(0, 3)
            load_xv_slice(2)
            load_xv_slice(3)
            for p in range(2):
                nc.sync.dma_start(wo_sb[:, p, :], wo[p])
            for g_ in range(TT // G):
                load_mask(1, g_)
            half_compute(xq_h0, wq_sb, 0, 0, QT)
            half_compute(xk_h0, wk_sb, 1, 0, KTs)
            # slot 0 of the attention pipeline runs here, its score tiles
            # borrowed from the proj psum ring, so the DVE/ACT engines start
            # ~16us before the h1 projection units clear the PE
            early_exprs = []
            msc0 = at.tile([128, G, SH], f16, tag="msc", bufs=2, name="msc_e0")
            expr0 = at.tile([128, G, SH], f16, tag="expr", bufs=3,
                            name="expr_e0")
            for i in range(G):
                sc = psp.tile([128, SH], f32, tag="proj", bufs=4,
                              name=f"sc_e0_{i}")
                for ch in range(2):
                    cs = slice(ch * 512, (ch + 1) * 512)
                    nc.tensor.matmul(
                        sc[:, cs],
                        KTs[0:64, 0, i * 128 : (i + 1) * 128],
                        QT[0:64, 0, ch * 512 : (ch + 1) * 512],
                        start=True,
                        stop=True,
                    )
                nc.vector.tensor_mul(
                    msc0[:, i, :], sc[:], mask_tiles[(0, 0)][:, i, :]
                )
            nc.scalar.activation(expr0[:], msc0[:], Exp, bias=bias_m2[:])
            early_exprs.append((0, 3, 0, 0, expr0, False))
            half_compute(xq_h1, wq_sb, 0, 1, QT)
            half_compute(xk_h1, wk_sb, 1, 1, KTs)
            psp_ctx.__exit__(None, None, None)
            stq_ctx.__exit__(None, None, None)

            # ---- Phase C/D: attention + output projection per s-half ----
            with (
                tc.tile_pool(name="ps_sc", bufs=2, space="PSUM") as pssc,
                tc.tile_pool(name="ps_u", bufs=2, space="PSUM") as psu,
            ):

                def emit_v_tt(tt):
                    # V in [t, e] orientation; psum borrowed from the sc pool
                    c, tl = divmod(tt, 4)
                    vps = pssc.tile([128, 256], f32, tag="sc", bufs=3)
                    for kp in range(KT // 2):
                        for i in range(2):
                            kt = 2 * kp + i
                            nc.tensor.matmul(
                                vps[:],
                                xstv[c][kp][:, i, tl * 128 : (tl + 1) * 128],
                                wv_sb[:, kt, :],
                                start=(kt == 0),
                                stop=False,
                            )
                    nc.tensor.matmul(
                        vps[:], ones_r[:, :], bv_sb[:, :], start=False, stop=True
                    )
                    nc.vector.tensor_copy(
                        V8[:, tt, :, 0:64],
                        vps[:, :].rearrange("p (h e) -> p h e", h=HPC),
                    )

                def attnv(u_ps, h, g, expr, offl_g=False):
                    # expr stationary per (q-tile, t-tile); [V | ones] moving.
                    # A matmul with start=True zeroes its whole 2KB psum bank,
                    # so each of the two 4-q-tile banks gets exactly one
                    # start (first MM) and one stop (last MM).
                    ua, ub = u_ps
                    iorder = (1, 2, 3, 0) if offl_g else tuple(range(G))
                    for qt in range(SH // 128):
                        u = ua if qt < 4 else ub
                        ql = qt % 4
                        for i in iorder:
                            tt = g * G + i
                            nc.tensor.matmul(
                                u[:, ql, 0:65],
                                expr[:, i, qt * 128 : (qt + 1) * 128],
                                V8[:, tt, h, 0:65],
                                start=(g == 0 and i == iorder[0] and ql == 0),
                                stop=(g == TT // G - 1 and i == iorder[-1]
                                      and ql == 3),
                            )

                ob_tiles = {}

                def phase_d_unit(sh, headsT, st_i, ch, tail=False):
                    # half-unit: 512 of the 1024 output columns of one s-tile;
                    # copies collect into a 2-s-tile group buffer so the out
                    # DMAs (625ns HWDGE enqueue each) are 4x fewer
                    s0 = sh * SH
                    cs = slice(ch * 512, (ch + 1) * 512)
                    o_ps = pssc.tile([128, 512], f32, tag="sc", bufs=3)
                    for p in range(2):
                        nc.tensor.matmul(
                            o_ps[:],
                            headsT[p][:, st_i * 128 : (st_i + 1) * 128],
                            wo_sb[:, p, cs],
                            start=(p == 0),
                            stop=(p == 1),
                        )
                    key = (sh, st_i // 2)
                    if key not in ob_tiles:
                        ob_tiles[key] = at.tile(
                            [128, 2, D], f16, tag="ob", bufs=2,
                            name=f"ob_{sh}_{st_i // 2}",
                        )
                    ob = ob_tiles[key]
                    if (2 * st_i + ch) % 2 == 0:
                        nc.scalar.copy(ob[:, st_i % 2, cs], o_ps[:])
                    else:
                        nc.vector.tensor_copy(ob[:, st_i % 2, cs], o_ps[:])
                    if st_i % 2 == 1 and ch == 1:
                        r0 = s0 + (st_i // 2) * 256
                        nc.sync.dma_start(
                            out[r0 : r0 + 256, :].rearrange(
                                "(q p) d -> p q d", p=128
                            ),
                            ob[:],
                        )

                def epilogue_b(sh, h, u_ps, u2, headsT):
                    # deferred half of the epilogue: second-bank normalize
                    # (+ its transposes), pushed one slot later to spread
                    # the DVE load at head boundaries
                    p, hh = divmod(h, 2)
                    ua, ub = u_ps
                    nrec = nrec_tiles[(sh, h)]
                    nc.vector.tensor_mul(
                        u2[p][:, 4:8, hh * 64 : hh * 64 + 64],
                        ub[:, :, 0:64],
                        nrec[:, 4:8, None].broadcast_to([128, 4, 64]),
                    )
                    if hh == 1:
                        for qt in range(4, SH // 128):
                            nc.sync.dma_start_transpose(
                                headsT[p][:, qt * 128 : (qt + 1) * 128],
                                u2[p][:, qt, :],
                            )

                def epilogue(sh, h, u_ps, u2, headsT, last=False):
                    # denominators -> one broadcast normalize -> transpose
                    p, hh = divmod(h, 2)
                    ua, ub = u_ps
                    nrec = at.tile([128, 8], f32, tag="nrec", bufs=2)
                    with nc.allow_low_precision(
                        "softmax denominators are O(100); f16-grade reciprocal "
                        "keeps 0.05% relative error"
                    ):
                        nc.vector.reciprocal(nrec[:, 0:4], ua[:, :, 64])
                        nc.vector.reciprocal(nrec[:, 4:8], ub[:, :, 64])
                    if hh == 0:
                        u2[p] = at.tile(
                            [128, 8, 128], f16, tag="u2", bufs=3,
                            name=f"u2_{sh}_{p}",
                        )
                    if last:
                        # per-q-tile normalize, PE transpose (the PE is idle
                        # here and DMA/HWDGE are the tail bottleneck), then
                        # that s-tile's full output projection immediately
                        for qt in range(SH // 128):
                            uq = (ua if qt < 4 else ub)[:, qt % 4, 0:64]
                            nc.vector.tensor_mul(
                                u2[p][:, qt, hh * 64 : hh * 64 + 64],
                                uq,
                                nrec[:, qt : qt + 1].broadcast_to([128, 64]),
                            )
                            tp = pssc.tile([128, 128], f16, tag="sc", bufs=3,
                                           name=f"tp_{qt}")
                            nc.tensor.transpose(
                                tp[:], u2[p][:, qt, :], ident128[:]
                            )
                            nc.vector.tensor_copy(
                                headsT[p][:, qt * 128 : (qt + 1) * 128], tp[:]
                            )
                            phase_d_unit(sh, headsT, qt, 0, tail=True)
                            phase_d_unit(sh, headsT, qt, 1, tail=True)
                        return
                    nrec_tiles[(sh, h)] = nrec
                    nc.vector.tensor_mul(
                        u2[p][:, 0:4, hh * 64 : hh * 64 + 64],
                        ua[:, :, 0:64],
                        nrec[:, 0:4, None].broadcast_to([128, 4, 64]),
                    )
                    if hh == 1:
                        for qt in range(4):
                            nc.sync.dma_start_transpose(
                                headsT[p][:, qt * 128 : (qt + 1) * 128],
                                u2[p][:, qt, :],
                            )

                pending = []
                nrec_tiles = {}
                for sh in range(2):
                    headsT = [
                        at.tile(
                            [128, SH], f16, tag="headsT", bufs=4, name=f"hT{sh}{pp}"
                        )
                        for pp in range(2)
                    ]
                    u2 = {}
                    u_tiles = {}
                    epi_b = []
                    av_q = list(early_exprs) if sh == 0 else []
                    # V projection t-tiles spread over sh0's early slots;
                    # h0's attn@V runs at lag 3 so tile tt is staged in time
                    vtt = {1: [0, 1, 2, 3], 2: [4, 5, 6], 3: [7, 8, 9],
                           4: [10, 11, 12], 5: [13, 14, 15]}
                    for slot in (range(1, HPC * (TT // G)) if sh == 0
                                 else range(HPC * (TT // G))):
                        h, g = divmod(slot, TT // G)
                        p, hh = divmod(h, 2)
                        er = slice(hh * 64, hh * 64 + 64)
                        offl = slot % 2 == 1
                        if h not in u_tiles:
                            u_tiles[h] = (
                                psu.tile([128, 4, 65], f32, tag="ua", bufs=1,
                                         name=f"ua_{sh}_{h}"),
                                psu.tile([128, 4, 65], f32, tag="ub", bufs=1,
                                         name=f"ub_{sh}_{h}"),
                            )
                        msc = at.tile([128, G, SH], f16, tag="msc", bufs=2)
                        expr = at.tile([128, G, SH], f16, tag="expr", bufs=3)
                        for i in range(G):
                            tt = g * G + i
                            sc = pssc.tile([128, SH], f32, tag="sc", bufs=3)
                            for ch in range(2):
                                cs = slice(ch * 512, (ch + 1) * 512)
                                nc.tensor.matmul(
                                    sc[:, cs],
                                    KTs[er, p, tt * 128 : (tt + 1) * 128],
                                    QT[
                                        er, p,
                                        sh * SH + ch * 512 : sh * SH + (ch + 1) * 512,
                                    ],
                                    start=True,
                                    stop=True,
                                )
                            mk = mask_tiles[(sh, g)]
                            if i == 0 and offl:
                                # DVE relief: ACT drains the psum to f16,
                                # Pool (SBUF-only on HW) applies the mask
                                sc_sb = at.tile([128, SH], f16, tag="scsb",
                                                bufs=2)
                                nc.scalar.copy(sc_sb[:], sc[:])
                                nc.gpsimd.tensor_mul(
                                    msc[:, i, :], sc_sb[:], mk[:, i, :]
                                )
                            else:
                                nc.vector.tensor_mul(
                                    msc[:, i, :], sc[:], mk[:, i, :]
                                )
                        if sh == 0:
                            for tt_ in vtt.get(slot, []):
                                emit_v_tt(tt_)
                        if offl:
                            # split exp: the DVE-masked tiles go first; the
                            # Pool-masked tile trails (attn@V lag absorbs it)
                            nc.scalar.activation(
                                expr[:, 1:4, :], msc[:, 1:4, :], Exp,
                                bias=bias_m2[:],
                            )
                            nc.scalar.activation(
                                expr[:, 0:1, :], msc[:, 0:1, :], Exp,
                                bias=bias_m2[:],
                            )
                        else:
                            nc.scalar.activation(
                                expr[:], msc[:], Exp, bias=bias_m2[:]
                            )
                        while epi_b:
                            epi_b.pop(0)()
                        av_q.append((slot, 3 if (sh == 0 and h == 0) else 2,
                                     h, g, expr, offl))
                        npop = 0
                        while (av_q and slot - av_q[0][0] >= av_q[0][1]
                               and npop < 2):
                            npop += 1
                            _, _, ah, ag, aexpr, aoffl = av_q.pop(0)
                            attnv(u_tiles[ah], ah, ag, aexpr, aoffl)
                            if ag == TT // G - 1:
                                epilogue(sh, ah, u_tiles[ah], u2, headsT)
                                epi_b.append(
                                    lambda sh=sh, ah=ah, u=u_tiles[ah],
                                    u2=u2, hT=headsT:
                                    epilogue_b(sh, ah, u, u2, hT)
                                )
                        if pending:
                            pending.pop(0)()
                    for _, _, ah, ag, aexpr, aoffl in av_q:
                        # drain leftover units into the exp-wait gaps
                        while pending:
                            pending.pop(0)()
                        attnv(u_tiles[ah], ah, ag, aexpr, aoffl)
                        if ag == TT // G - 1:
                            epilogue(sh, ah, u_tiles[ah], u2, headsT,
                                     last=(sh == 1))
                            if sh == 0:
                                epilogue_b(sh, ah, u_tiles[ah], u2, headsT)
                    pending = [
                        (lambda sh=sh, headsT=headsT, st_i=st_i, ch=ch:
                         phase_d_unit(sh, headsT, st_i, ch))
                        for st_i in range(SH // 128)
                        for ch in range(2)
                    ] if sh == 0 else []

    nc.finalize()
    return nc


def kernel(q, k, v, mask, W_q, b_q, W_k, b_k, W_v, b_v, W_o, b_o):
    from concourse.bass_utils import run_bass_kernel_spmd

    q = np.asarray(q, dtype=np.float32)
    k = np.asarray(k, dtype=np.float32)
    v = np.asarray(v, dtype=np.float32)
    mask = np.asarray(mask, dtype=np.float32)
    W_q = np.asarray(W_q, dtype=np.float32)
    b_q = np.asarray(b_q, dtype=np.float32)
    W_k = np.asarray(W_k, dtype=np.float32)
    b_k = np.asarray(b_k, dtype=np.float32)
    W_v = np.asarray(W_v, dtype=np.float32)
    b_v = np.asarray(b_v, dtype=np.float32)
    W_o = np.asarray(W_o, dtype=np.float32)
    b_o = np.asarray(b_o, dtype=np.float32)

    if "nc" not in _cache:
        _cache["nc"] = _build_program()
    nc = _cache["nc"]

    scale = 1.0 / np.sqrt(np.float32(DH))
    maskTh = np.ascontiguousarray((mask.T * scale).astype(np.float16))

    def xT16(x_b):  # [S, D] -> [D, S] f16
        return np.ascontiguousarray(x_b.T).astype(np.float16)

    def w16(W, heads):  # [H, D, DH] -> [2, KT, 128, 128] f16
        cols = []
        for pp in range(2):
            h0, h1 = heads[2 * pp], heads[2 * pp + 1]
            wpair = np.concatenate([W[h0], W[h1]], axis=1)  # [D, 128]
            cols.append(wpair.reshape(KT, 128, 128))
        return np.ascontiguousarray(np.stack(cols, axis=0)).astype(np.float16)

    def wv16(W, heads):  # [H, D, DH] -> [KT, 128, 256] f16
        wcat = np.concatenate([W[h] for h in heads], axis=1)  # [D, 256]
        return np.ascontiguousarray(wcat.reshape(KT, 128, 256)).astype(np.float16)

    def bcat(bvec, heads):  # [H, DH] -> [128, 2] f32 (pair-concat per column)
        return np.stack(
            [
                np.concatenate([bvec[heads[2 * pp]], bvec[heads[2 * pp + 1]]])
                for pp in range(2)
            ],
            axis=1,
        ).astype(np.float32)

    in_maps = []
    for c in range(NCORES):
        b, g = divmod(c, HPC)
        heads = list(range(HPC * g, HPC * g + HPC))
        in_maps.append(
            {
                "xq": xT16(q[b]),
                "xk": xT16(k[b]),
                "xv": xT16(v[b]),
                "maskT": maskTh,
                "wq": w16(W_q, heads),
                "wk": w16(W_k, heads),
                "wv": wv16(W_v, heads),
                "bvrow": np.ascontiguousarray(
                    np.concatenate([b_v[h] for h in heads])[None, :]
                ).astype(np.float16),
                "bqkv": np.ascontiguousarray(
                    np.concatenate(
                        [bcat(b_q, heads), bcat(b_k, heads), bcat(b_v, heads)],
                        axis=1,
                    )
                ),
                "wo": np.ascontiguousarray(
                    W_o[heads[0] * DH : (heads[-1] + 1) * DH].reshape(2, 128, D)
                ).astype(np.float16),
            }
        )

    trace = bool(int(os.environ.get("KERNEL_TRACE", "0")))
    res = run_bass_kernel_spmd(nc, in_maps, list(range(NCORES)), trace=trace)
    _cache["last_results"] = res

    full = np.zeros((B, S, D), np.float32)
    for c in range(NCORES):
        full[c // HPC] += np.asarray(res.results[c]["out"], dtype=np.float32)
    full += b_o[None, None, :]
    return full


# revision 93
# speedup vs baseline: 1.0468x; 1.0000x over previous
"""Multi-head attention on 8 TRN2 NeuronCores.

Sharding: core c handles batch b = c // 4 and heads [4g, 4g+4) with g = c % 4.
Each core computes its 4 heads' contribution to out[b] = concat(heads) @ W_o;
the host sums the 4 per-batch partials and adds b_o.

v9 dataflow (per core), f16 value path:
  - attn@V operand swap: expr (exp'd scores, [k, q] layout) is the STATIONARY
    matmul operand per (q-tile, k-tile); V (+ a ones column for the softmax
    denominators) is the 65-wide MOVING operand, cutting attn@V PE time ~8x
    (the PE charges by moving free size only).  u psum is two bank-aligned
    [q=128, 4 q-tiles, 65] tiles per head (a start=True matmul zeroes its
    whole 2KB psum bank, so each bank gets exactly one start/stop pair).
  - flat 16-slot pipeline per s-half (slot = one 4-t-tile softmax group);
    attn@V trails its group's exp by 2-3 slots so the in-order PE never
    stalls on the ACT; o-projection half-units and the V projection
    (spread 2-3 t-tiles per early slot, xv loaded in 512-column slices)
    fill the PE's leftover capacity.
  - GPSIMD cannot touch PSUM on real HW, so psum drains are DVE/ACT only:
    mask-muls on the DVE, except one tile on alternating slots routed
    ACT-copy -> Pool-multiply (with that slot's exp split 3+1 so the slower
    Pool path trails off the critical chain).  Normalize = DVE reciprocal of
    u column 64 + broadcast-AP multiply; DMA xbar transposes ([128,128] per
    q-tile, 2 heads packed) produce the [e, q] layout for the o-projection.
  - last head fuses per-q-tile: normalize -> PE transpose (identity matmul)
    -> o-projection immediately, so the tail pipelines instead of
    serializing; o-projection copies collect into 2-s-tile group buffers
    (one out-DMA per 256 rows, 4x fewer 625ns HWDGE enqueues).
  - Q/K projected in column-half units against a 4-buffer [128,1024] proj
    psum ring, so the first scores wait on ~5MB of input DMA instead of
    9MB; attention slot 0 is emitted right after the K-h0 unit with its
    score tiles borrowed from that same ring, starting the DVE/ACT
    pipeline ~16us before the h1 units clear the in-order PE.
  - explicit SP-queue DMA priority: wq, xq-h0, wk, xk-h0, mask g0, x h1
    halves, mask g1, wv, xv c0, mask g2, xv c1, mask g3, xv c2/c3, wo,
    mask half 1; warm-up matmuls ramp the PE p-state through the initial
    DMA wait.
"""

import os
import numpy as np

B = 2
S = 2048
D = 1024
H = 16
DH = 64
NCORES = 8
HPC = 4  # heads per core
SH = S // 2  # s-half processed per attention sweep
TT = S // 128  # 16 t-tiles
KT = 8  # k-tiles in the contraction (1024 = 8 * 128)
G = 4  # t-tiles per softmax group (exp FD = G*1024)

_cache = {}


def _build_program():
    import concourse.mybir as mybir
    import concourse.tile as tile
    from concourse import bacc

    f32 = mybir.dt.float32
    f16 = mybir.dt.float16
    Exp = mybir.ActivationFunctionType.Exp
    Ident = mybir.ActivationFunctionType.Identity
    Alu = mybir.AluOpType

    nc = bacc.Bacc(None, target_bir_lowering=False, debug=False)
    xq = nc.declare_dram_parameter("xq", [D, S], f16, isOutput=False)
    xk = nc.declare_dram_parameter("xk", [D, S], f16, isOutput=False)
    xv = nc.declare_dram_parameter("xv", [D, S], f16, isOutput=False)
    maskT = nc.declare_dram_parameter("maskT", [S, S], f16, isOutput=False)
    wq = nc.declare_dram_parameter("wq", [2, KT, 128, 128], f16, isOutput=False)
    wk = nc.declare_dram_parameter("wk", [2, KT, 128, 128], f16, isOutput=False)
    wv = nc.declare_dram_parameter("wv", [KT, 128, 256], f16, isOutput=False)
    bqkv = nc.declare_dram_parameter("bqkv", [128, 6], f32, isOutput=False)
    bvrow = nc.declare_dram_parameter("bvrow", [1, 256], f16, isOutput=False)
    wo = nc.declare_dram_parameter("wo", [2, 128, D], f16, isOutput=False)
    out = nc.declare_dram_parameter("out", [S, D], f16, isOutput=True)

    with tile.TileContext(nc) as tc:
        with (
            tc.tile_pool(name="persist", bufs=1) as pw,
            tc.tile_pool(name="stage", bufs=1) as st,
            tc.tile_pool(name="attn", bufs=2) as at,
        ):
            wq_sb = pw.tile([128, 2, KT, 128], f16, tag="wq_sb")
            wk_sb = pw.tile([128, 2, KT, 128], f16, tag="wk_sb")
            wv_sb = pw.tile([128, KT, 256], f16, tag="wv_sb")
            bq_sb = pw.tile([128, 6], f32, tag="bq_sb")
            wo_sb = pw.tile([128, 2, D], f16, tag="wo_sb")
            ones_r = pw.tile([1, 128], f16, tag="ones_r")
            bv_sb = pw.tile([1, 256], f16, tag="bv_sb")
            bias_m2 = pw.tile([128, 1], f32, tag="bias_m2")
            warm_rhs = pw.tile([1, 512], f16, tag="warm_rhs")
            ident128 = pw.tile([128, 128], f16, tag="ident128")
            nc.gpsimd.memset(bias_m2[:], -2.0)
            nc.gpsimd.memset(ones_r[:], 1.0)
            nc.gpsimd.memset(warm_rhs[:], 1.0)
            from concourse.masks import make_identity
            make_identity(nc, ident128[:])
            QT = pw.tile([128, 2, S], f16, tag="QT", name="QT")
            KTs = pw.tile([128, 2, S], f16, tag="KTs", name="KTs")
            V8 = pw.tile([128, TT, HPC, 65], f16, tag="V8")
            for h_ in range(HPC):
                nc.gpsimd.memset(V8[:, :, h_, 64:65], 1.0)
            mask_tiles = {}

            # ---- weight/bias loads; wq first so Q projection starts ASAP
            for p in range(2):
                nc.sync.dma_start(
                    wq_sb[:, p, :, :], wq[p].rearrange("kt p m -> p kt m")
                )
            nc.sync.dma_start(bq_sb[:, :], bqkv[:, :])
            nc.sync.dma_start(bv_sb[:, :], bvrow[:, :])

            psp_ctx = tc.tile_pool(name="ps_proj", bufs=2, space="PSUM")
            psp = psp_ctx.__enter__()
            # dependency-free warm-up matmuls: ramp the PE p-state during the
            # initial DMA wait so projections run at full clock
            warm_ps = psp.tile([128, SH], f32, tag="proj", bufs=4,
                               name="warm_ps")
            for wi in range(14):
                nc.tensor.matmul(
                    warm_ps[:, 0:512], ones_r[:, :], warm_rhs[:, :],
                    start=True, stop=True,
                )
            stq_ctx = tc.tile_pool(name="qkstage", bufs=1)
            stq = stq_ctx.__enter__()

            def half_dmas(x_dram, wbi, half):
                xh = [
                    stq.tile([128, 2, SH], f16, tag="xst", bufs=8,
                             name=f"x{wbi}{half}_{kp}")
                    for kp in range(KT // 2)
                ]
                for kp in range(KT // 2):
                    nc.sync.dma_start(
                        xh[kp][:],
                        x_dram[
                            kp * 256 : (kp + 1) * 256,
                            half * SH : (half + 1) * SH,
                        ].rearrange("(i p) s -> p i s", p=128),
                    )
                return xh

            def half_compute(xh, w_sb, wbi, half, dst):
                accs = [
                    psp.tile([128, SH], f32, tag="proj", bufs=4,
                             name=f"acc{wbi}{half}{pp}")
                    for pp in range(2)
                ]
                for p in range(2):
                    for kp in range(KT // 2):
                        for i in range(2):
                            kt = 2 * kp + i
                            for ch in range(2):
                                cs = slice(ch * 512, (ch + 1) * 512)
                                nc.tensor.matmul(
                                    accs[p][:, cs],
                                    w_sb[:, p, kt, :],
                                    xh[kp][:, i, cs],
                                    start=(kt == 0),
                                    stop=(kt == KT - 1),
                                )
                hs = slice(half * SH, (half + 1) * SH)
                for pp in range(2):
                    nc.scalar.activation(
                        dst[:, pp, hs], accs[pp][:], Ident,
                        bias=bq_sb[:, 2 * wbi + pp : 2 * wbi + pp + 1],
                        scale=1.0,
                    )

            # ---- Phase B: Q/K in column-half units so the first scores
            # wait on ~5MB of input DMA instead of 9MB; the x stage ring
            # holds both tensors' h0 halves at once (8 x 4KB).
            def load_mask(sh, g_):
                mt = at.tile(
                    [128, G, SH], f16, tag="mask", bufs=4, name=f"mask{sh}_{g_}"
                )
                mask_tiles[(sh, g_)] = mt
                nc.sync.dma_start(
                    mt[:],
                    maskT[
                        g_ * G * 128 : (g_ + 1) * G * 128, sh * SH : (sh + 1) * SH
                    ].rearrange("(tt p) s -> p tt s", p=128),
                )

            xstv = [
                [
                    st.tile([128, 2, 512], f16, tag="xstv", bufs=16,
                            name=f"xstv{c}_{kp}")
                    for kp in range(KT // 2)
                ]
                for c in range(4)
            ]

            def load_xv_slice(c):
                for kp in range(KT // 2):
                    nc.sync.dma_start(
                        xstv[c][kp][:],
                        xv[kp * 256 : (kp + 1) * 256, c * 512 : (c + 1) * 512]
                        .rearrange("(i p) s -> p i s", p=128),
                    )

            xq_h0 = half_dmas(xq, 0, 0)
            for p in range(2):  # prefetch K weights behind the xq-h0 stages
                nc.sync.dma_start(
                    wk_sb[:, p, :, :], wk[p].rearrange("kt p m -> p kt m")
                )
            xk_h0 = half_dmas(xk, 1, 0)
            load_mask(0, 0)
            xq_h1 = half_dmas(xq, 0, 1)
            xk_h1 = half_dmas(xk, 1, 1)
            load_mask(0, 1)
            nc.sync.dma_start(wv_sb[:, :, :], wv[:].rearrange("kt k e -> k kt e"))
            load_xv_slice(0)
            load_mask(0, 2)
            load_xv_slice(1)
            load_mask(0, 3)
            load_xv_slice(2)
            load_xv_slice(3)
            for p in range(2):
                nc.sync.dma_start(wo_sb[:, p, :], wo[p])
            for g_ in range(TT // G):
                load_mask(1, g_)
            half_compute(xq_h0, wq_sb, 0, 0, QT)
            half_compute(xk_h0, wk_sb, 1, 0, KTs)
            # slot 0 of the attention pipeline runs here, its score tiles
            # borrowed from the proj psum ring, so the DVE/ACT engines start
            # ~16us before the h1 projection units clear the PE
            early_exprs = []
            msc0 = at.tile([128, G, SH], f16, tag="msc", bufs=2, name="msc_e0")
            expr0 = at.tile([128, G, SH], f16, tag="expr", bufs=3,
                            name="expr_e0")
            for i in range(G):
                sc = psp.tile([128, SH], f32, tag="proj", bufs=4,
                              name=f"sc_e0_{i}")
                for ch in range(2):
                    cs = slice(ch * 512, (ch + 1) * 512)
                    nc.tensor.matmul(
                        sc[:, cs],
                        KTs[0:64, 0, i * 128 : (i + 1) * 128],
                        QT[0:64, 0, ch * 512 : (ch + 1) * 512],
                        start=True,
                        stop=True,
                    )
                if i == 0:
                    sc_sb0 = at.tile([128, SH], f16, tag="scsb", bufs=2,
                                     name="scsb_e0")
                    nc.scalar.copy(sc_sb0[:], sc[:])
                    nc.gpsimd.tensor_mul(
                        msc0[:, i, :], sc_sb0[:], mask_tiles[(0, 0)][:, i, :]
                    )
                else:
                    nc.vector.tensor_mul(
                        msc0[:, i, :], sc[:], mask_tiles[(0, 0)][:, i, :]
                    )
            nc.scalar.activation(
                expr0[:, 1:4, :], msc0[:, 1:4, :], Exp, bias=bias_m2[:]
            )
            nc.scalar.activation(
                expr0[:, 0:1, :], msc0[:, 0:1, :], Exp, bias=bias_m2[:]
            )
            early_exprs.append((0, 4, 0, 0, expr0, True))
            half_compute(xq_h1, wq_sb, 0, 1, QT)
            half_compute(xk_h1, wk_sb, 1, 1, KTs)
            psp_ctx.__exit__(None, None, None)
            stq_ctx.__exit__(None, None, None)

            # ---- Phase C/D: attention + output projection per s-half ----
            with (
                tc.tile_pool(name="ps_sc", bufs=2, space="PSUM") as pssc,
                tc.tile_pool(name="ps_u", bufs=2, space="PSUM") as psu,
            ):

                def emit_v_tt(tt):
                    # V in [t, e] orientation; psum borrowed from the sc pool
                    c, tl = divmod(tt, 4)
                    vps = pssc.tile([128, 256], f32, tag="sc", bufs=3)
                    for kp in range(KT // 2):
                        for i in range(2):
                            kt = 2 * kp + i
                            nc.tensor.matmul(
                                vps[:],
                                xstv[c][kp][:, i, tl * 128 : (tl + 1) * 128],
                                wv_sb[:, kt, :],
                                start=(kt == 0),
                                stop=False,
                            )
                    nc.tensor.matmul(
                        vps[:], ones_r[:, :], bv_sb[:, :], start=False, stop=True
                    )
                    nc.vector.tensor_copy(
                        V8[:, tt, :, 0:64],
                        vps[:, :].rearrange("p (h e) -> p h e", h=HPC),
                    )

                def attnv(u_ps, h, g, expr, offl_g=False):
                    # expr stationary per (q-tile, t-tile); [V | ones] moving.
                    # A matmul with start=True zeroes its whole 2KB psum bank,
                    # so each of the two 4-q-tile banks gets exactly one
                    # start (first MM) and one stop (last MM).
                    ua, ub = u_ps
                    iorder = (1, 2, 3, 0) if offl_g else tuple(range(G))
                    for qt in range(SH // 128):
                        u = ua if qt < 4 else ub
                        ql = qt % 4
                        for i in iorder:
                            tt = g * G + i
                            nc.tensor.matmul(
                                u[:, ql, 0:65],
                                expr[:, i, qt * 128 : (qt + 1) * 128],
                                V8[:, tt, h, 0:65],
                                start=(g == 0 and i == iorder[0] and ql == 0),
                                stop=(g == TT // G - 1 and i == iorder[-1]
                                      and ql == 3),
                            )

                ob_tiles = {}

                def phase_d_unit(sh, headsT, st_i, ch, tail=False):
                    # half-unit: 512 of the 1024 output columns of one s-tile;
                    # copies collect into a 2-s-tile group buffer so the out
                    # DMAs (625ns HWDGE enqueue each) are 4x fewer
                    s0 = sh * SH
                    cs = slice(ch * 512, (ch + 1) * 512)
                    o_ps = pssc.tile([128, 512], f32, tag="sc", bufs=3)
                    for p in range(2):
                        nc.tensor.matmul(
                            o_ps[:],
                            headsT[p][:, st_i * 128 : (st_i + 1) * 128],
                            wo_sb[:, p, cs],
                            start=(p == 0),
                            stop=(p == 1),
                        )
                    key = (sh, st_i // 2)
                    if key not in ob_tiles:
                        ob_tiles[key] = at.tile(
                            [128, 2, D], f16, tag="ob", bufs=2,
                            name=f"ob_{sh}_{st_i // 2}",
                        )
                    ob = ob_tiles[key]
                    if (2 * st_i + ch) % 2 == 0:
                        nc.scalar.copy(ob[:, st_i % 2, cs], o_ps[:])
                    else:
                        nc.vector.tensor_copy(ob[:, st_i % 2, cs], o_ps[:])
                    if st_i % 2 == 1 and ch == 1:
                        r0 = s0 + (st_i // 2) * 256
                        nc.sync.dma_start(
                            out[r0 : r0 + 256, :].rearrange(
                                "(q p) d -> p q d", p=128
                            ),
                            ob[:],
                        )

                def epilogue_b(sh, h, u_ps, u2, headsT):
                    # deferred half of the epilogue: second-bank normalize
                    # (+ its transposes), pushed one slot later to spread
                    # the DVE load at head boundaries
                    p, hh = divmod(h, 2)
                    ua, ub = u_ps
                    nrec = nrec_tiles[(sh, h)]
                    nc.vector.tensor_mul(
                        u2[p][:, 4:8, hh * 64 : hh * 64 + 64],
                        ub[:, :, 0:64],
                        nrec[:, 4:8, None].broadcast_to([128, 4, 64]),
                    )
                    if hh == 1:
                        for qt in range(4, SH // 128):
                            nc.sync.dma_start_transpose(
                                headsT[p][:, qt * 128 : (qt + 1) * 128],
                                u2[p][:, qt, :],
                            )

                def epilogue(sh, h, u_ps, u2, headsT, last=False):
                    # denominators -> one broadcast normalize -> transpose
                    p, hh = divmod(h, 2)
                    ua, ub = u_ps
                    nrec = at.tile([128, 8], f32, tag="nrec", bufs=2)
                    with nc.allow_low_precision(
                        "softmax denominators are O(100); f16-grade reciprocal "
                        "keeps 0.05% relative error"
                    ):
                        nc.vector.reciprocal(nrec[:, 0:4], ua[:, :, 64])
                        nc.vector.reciprocal(nrec[:, 4:8], ub[:, :, 64])
                    if hh == 0:
                        u2[p] = at.tile(
                            [128, 8, 128], f16, tag="u2", bufs=3,
                            name=f"u2_{sh}_{p}",
                        )
                    if last:
                        # per-q-tile normalize, PE transpose (the PE is idle
                        # here and DMA/HWDGE are the tail bottleneck), then
                        # that s-tile's full output projection immediately
                        for qt in range(SH // 128):
                            uq = (ua if qt < 4 else ub)[:, qt % 4, 0:64]
                            nc.vector.tensor_mul(
                                u2[p][:, qt, hh * 64 : hh * 64 + 64],
                                uq,
                                nrec[:, qt : qt + 1].broadcast_to([128, 64]),
                            )
                            tp = pssc.tile([128, 128], f16, tag="sc", bufs=3,
                                           name=f"tp_{qt}")
                            nc.tensor.transpose(
                                tp[:], u2[p][:, qt, :], ident128[:]
                            )
                            nc.vector.tensor_copy(
                                headsT[p][:, qt * 128 : (qt + 1) * 128], tp[:]
                            )
                            phase_d_unit(sh, headsT, qt, 0, tail=True)
                            phase_d_unit(sh, headsT, qt, 1, tail=True)
                        return
                    nrec_tiles[(sh, h)] = nrec
                    nc.vector.tensor_mul(
                        u2[p][:, 0:4, hh * 64 : hh * 64 + 64],
                        ua[:, :, 0:64],
                        nrec[:, 0:4, None].broadcast_to([128, 4, 64]),
                    )
                    if hh == 1:
                        for qt in range(4):
                            nc.sync.dma_start_transpose(
                                headsT[p][:, qt * 128 : (qt + 1) * 128],
                                u2[p][:, qt, :],
                            )

                pending = []
                nrec_tiles = {}
                for sh in range(2):
                    headsT = [
                        at.tile(
                            [128, SH], f16, tag="headsT", bufs=4, name=f"hT{sh}{pp}"
                        )
                        for pp in range(2)
                    ]
                    u2 = {}
                    u_tiles = {}
                    epi_b = []
                    av_q = list(early_exprs) if sh == 0 else []
                    # V projection t-tiles spread over sh0's early slots;
                    # h0's attn@V runs at lag 3 so tile tt is staged in time
                    vtt = {1: [0, 1, 2, 3], 2: [4, 5, 6], 3: [7, 8, 9],
                           4: [10, 11, 12], 5: [13, 14, 15]}
                    for slot in (range(1, HPC * (TT // G)) if sh == 0
                                 else range(HPC * (TT // G))):
                        h, g = divmod(slot, TT // G)
                        p, hh = divmod(h, 2)
                        er = slice(hh * 64, hh * 64 + 64)
                        offl = slot % 2 == 1
                        if h not in u_tiles:
                            u_tiles[h] = (
                                psu.tile([128, 4, 65], f32, tag="ua", bufs=1,
                                         name=f"ua_{sh}_{h}"),
                                psu.tile([128, 4, 65], f32, tag="ub", bufs=1,
                                         name=f"ub_{sh}_{h}"),
                            )
                        msc = at.tile([128, G, SH], f16, tag="msc", bufs=2)
                        expr = at.tile([128, G, SH], f16, tag="expr", bufs=3)
                        for i in range(G):
                            tt = g * G + i
                            sc = pssc.tile([128, SH], f32, tag="sc", bufs=3)
                            for ch in range(2):
                                cs = slice(ch * 512, (ch + 1) * 512)
                                nc.tensor.matmul(
                                    sc[:, cs],
                                    KTs[er, p, tt * 128 : (tt + 1) * 128],
                                    QT[
                                        er, p,
                                        sh * SH + ch * 512 : sh * SH + (ch + 1) * 512,
                                    ],
                                    start=True,
                                    stop=True,
                                )
                            mk = mask_tiles[(sh, g)]
                            if i == 0 and offl:
                                # DVE relief: ACT drains the psum to f16,
                                # Pool (SBUF-only on HW) applies the mask
                                sc_sb = at.tile([128, SH], f16, tag="scsb",
                                                bufs=2)
                                nc.scalar.copy(sc_sb[:], sc[:])
                                nc.gpsimd.tensor_mul(
                                    msc[:, i, :], sc_sb[:], mk[:, i, :]
                                )
                            else:
                                nc.vector.tensor_mul(
                                    msc[:, i, :], sc[:], mk[:, i, :]
                                )
                        if sh == 0:
                            for tt_ in vtt.get(slot, []):
                                emit_v_tt(tt_)
                        if offl:
                            # split exp: the DVE-masked tiles go first; the
                            # Pool-masked tile trails (attn@V lag absorbs it)
                            nc.scalar.activation(
                                expr[:, 1:4, :], msc[:, 1:4, :], Exp,
                                bias=bias_m2[:],
                            )
                            nc.scalar.activation(
                                expr[:, 0:1, :], msc[:, 0:1, :], Exp,
                                bias=bias_m2[:],
                            )
                        else:
                            nc.scalar.activation(
                                expr[:], msc[:], Exp, bias=bias_m2[:]
                            )
                        while epi_b:
                            epi_b.pop(0)()
                        av_q.append((slot, 3 if (sh == 0 and h == 0) else 2,
                                     h, g, expr, offl))
                        npop = 0
                        while (av_q and slot - av_q[0][0] >= av_q[0][1]
                               and npop < 2):
                            npop += 1
                            _, _, ah, ag, aexpr, aoffl = av_q.pop(0)
                            attnv(u_tiles[ah], ah, ag, aexpr, aoffl)
                            if ag == TT // G - 1:
                                epilogue(sh, ah, u_tiles[ah], u2, headsT)
                                epi_b.append(
                                    lambda sh=sh, ah=ah, u=u_tiles[ah],
                                    u2=u2, hT=headsT:
                                    epilogue_b(sh, ah, u, u2, hT)
                                )
                        if pending:
                            pending.pop(0)()
                    for _, _, ah, ag, aexpr, aoffl in av_q:
                        # drain leftover units into the exp-wait gaps
                        while pending:
                            pending.pop(0)()
                        attnv(u_tiles[ah], ah, ag, aexpr, aoffl)
                        if ag == TT // G - 1:
                            epilogue(sh, ah, u_tiles[ah], u2, headsT,
                                     last=(sh == 1))
                            if sh == 0:
                                epilogue_b(sh, ah, u_tiles[ah], u2, headsT)
                    pending = [
                        (lambda sh=sh, headsT=headsT, st_i=st_i, ch=ch:
                         phase_d_unit(sh, headsT, st_i, ch))
                        for st_i in range(SH // 128)
                        for ch in range(2)
                    ] if sh == 0 else []

    nc.finalize()
    return nc


def kernel(q, k, v, mask, W_q, b_q, W_k, b_k, W_v, b_v, W_o, b_o):
    from concourse.bass_utils import run_bass_kernel_spmd

    q = np.asarray(q, dtype=np.float32)
    k = np.asarray(k, dtype=np.float32)
    v = np.asarray(v, dtype=np.float32)
    mask = np.asarray(mask, dtype=np.float32)
    W_q = np.asarray(W_q, dtype=np.float32)
    b_q = np.asarray(b_q, dtype=np.float32)
    W_k = np.asarray(W_k, dtype=np.float32)
    b_k = np.asarray(b_k, dtype=np.float32)
    W_v = np.asarray(W_v, dtype=np.float32)
    b_v = np.asarray(b_v, dtype=np.float32)
    W_o = np.asarray(W_o, dtype=np.float32)
    b_o = np.asarray(b_o, dtype=np.float32)

    if "nc" not in _cache:
        _cache["nc"] = _build_program()
    nc = _cache["nc"]

    scale = 1.0 / np.sqrt(np.float32(DH))
    maskTh = np.ascontiguousarray((mask.T * scale).astype(np.float16))

    def xT16(x_b):  # [S, D] -> [D, S] f16
        return np.ascontiguousarray(x_b.T).astype(np.float16)

    def w16(W, heads):  # [H, D, DH] -> [2, KT, 128, 128] f16
        cols = []
        for pp in range(2):
            h0, h1 = heads[2 * pp], heads[2 * pp + 1]
            wpair = np.concatenate([W[h0], W[h1]], axis=1)  # [D, 128]
            cols.append(wpair.reshape(KT, 128, 128))
        return np.ascontiguousarray(np.stack(cols, axis=0)).astype(np.float16)

    def wv16(W, heads):  # [H, D, DH] -> [KT, 128, 256] f16
        wcat = np.concatenate([W[h] for h in heads], axis=1)  # [D, 256]
        return np.ascontiguousarray(wcat.reshape(KT, 128, 256)).astype(np.float16)

    def bcat(bvec, heads):  # [H, DH] -> [128, 2] f32 (pair-concat per column)
        return np.stack(
            [
                np.concatenate([bvec[heads[2 * pp]], bvec[heads[2 * pp + 1]]])
                for pp in range(2)
            ],
            axis=1,
        ).astype(np.float32)

    in_maps = []
    for c in range(NCORES):
        b, g = divmod(c, HPC)
        heads = list(range(HPC * g, HPC * g + HPC))
        in_maps.append(
            {
                "xq": xT16(q[b]),
                "xk": xT16(k[b]),
                "xv": xT16(v[b]),
                "maskT": maskTh,
                "wq": w16(W_q, heads),
                "wk": w16(W_k, heads),
                "wv": wv16(W_v, heads),
                "bvrow": np.ascontiguousarray(
                    np.concatenate([b_v[h] for h in heads])[None, :]
                ).astype(np.float16),
                "bqkv": np.ascontiguousarray(
                    np.concatenate(
                        [bcat(b_q, heads), bcat(b_k, heads), bcat(b_v, heads)],
                        axis=1,
                    )
                ),
                "wo": np.ascontiguousarray(
                    W_o[heads[0] * DH : (heads[-1] + 1) * DH].reshape(2, 128, D)
                ).astype(np.float16),
            }
        )

    trace = bool(int(os.environ.get("KERNEL_TRACE", "0")))
    res = run_bass_kernel_spmd(nc, in_maps, list(range(NCORES)), trace=trace)
    _cache["last_results"] = res

    full = np.zeros((B, S, D), np.float32)
    for c in range(NCORES):
        full[c // HPC] += np.asarray(res.results[c]["out"], dtype=np.float32)
    full += b_o[None, None, :]
    return full
